# revision 4
# baseline (speedup 1.0000x reference)
"""Trainium2 Bass kernel for nn_CosSimRouter_learn_49778670960796.

Host: cosine-similarity scoring / sort / gather (tiny, shape-determining).
Device (8 NeuronCores, tensor-parallel over heads/hidden):
  3x MHA + FFN + logits; fp16 storage + matmuls (fp32 PSUM accum).
  Comm/compute overlap:
   - MHA1 (small) first; its AllReduce overlaps MHA2 (large).
   - AR2/ARc split into 2 feature-half chunks, issued as soon as their
     out-proj half is staged.
   - The consumers of AR2/ARc (cross-attn q-proj, FFN f1) use the
     affine-LN identity  W@LN(y) = (W@y + rowsum(W) (x) (-mean)) * rstd
     so their heavy weight sweeps consume un-normalized y chunk by chunk
     while the AllReduce is still in flight.
   - FFN2 output: fp16 ReduceScatter (2 chunks) + tiny stat AllReduce;
     logits from the affine-LN identity.
Host: top-k + final gather (exact rows of the input).
"""

import numpy as np

E = 4096
H = 16
HID = 8192
GAMMA = 0.2
TEMP = 0.05
EXPAND = 0.7
NCORES = 8
ET = E // 128  # 32 feature tiles
DH = E // H  # 256
HL = H // NCORES  # 2 heads per core
DLOC = HL * DH  # 512 local head dims
FLOC = HID // NCORES  # 1024 local ffn hidden
NCH = 2  # feature-half chunks per big collective
CROWS = E // NCH  # 2048
KC = ET // NCH  # 16 feature tiles per chunk

_CACHE = {}


# ----------------------------------------------------------------------------
# host-side reference math (numpy, fp32) for the scoring stage + fallback
# ----------------------------------------------------------------------------

def _score_partition(vision_feature, text_embed, attention_mask):
    vf = vision_feature.astype(np.float32)
    te = text_embed.astype(np.float32)
    vn = vf / np.maximum(np.linalg.norm(vf, axis=-1, keepdims=True), 1e-8)
    tn = te / np.maximum(np.linalg.norm(te, axis=-1, keepdims=True), 1e-8)
    cs = vn @ tn.T
    cs = np.where(attention_mask[None, :], cs, np.float32(0.0))
    m = cs.max(axis=-1) / np.float32(TEMP)
    e = np.exp(m - m.max())
    scores = e / e.sum()
    order = np.argsort(-scores, kind="stable")
    cum = np.cumsum(scores[order])
    t = int((cum <= GAMMA).sum())
    return t, order[:t], order[t:]


def _ln_np(x):
    m = x.mean(-1, keepdims=True)
    v = ((x - m) ** 2).mean(-1, keepdims=True)
    return (x - m) / np.sqrt(v + 1e-5)


def _gelu_np(x):
    import math

    erf = np.frompyfunc(math.erf, 1, 1)
    return (x * 0.5 * (1.0 + erf(x / math.sqrt(2.0)).astype(np.float64))
            ).astype(x.dtype)


def _mha_np(q_in, kv_in, Wqkv, bqkv, Wo, bo):
    dh = E // H
    Wq, Wk, Wv = np.split(Wqkv, 3, axis=0)
    bq, bk, bv = np.split(bqkv, 3)
    q = (q_in @ Wq.T + bq).reshape(-1, H, dh)
    k = (kv_in @ Wk.T + bk).reshape(-1, H, dh)
    v = (kv_in @ Wv.T + bv).reshape(-1, H, dh)
    att = np.einsum("qhd,khd->hqk", q, k) / np.float32(np.sqrt(dh))
    att = att - att.max(-1, keepdims=True)
    att = np.exp(att)
    att /= att.sum(-1, keepdims=True)
    o = np.einsum("hqk,khd->qhd", att.astype(np.float32), v).reshape(-1, E)
    return o @ Wo.T + bo


def _reference_np(vision_feature, text_embed, attention_mask,
                  Wqkv1, bqkv1, Wo1, bo1, Wqkv2, bqkv2, Wo2, bo2,
                  Wqkvc, bqkvc, Woc, boc, Wf1, bf1, Wf2, bf2, Ws, bs):
    t, sel_idx, rem_idx = _score_partition(vision_feature, text_embed,
                                           attention_mask)
    sel = vision_feature[sel_idx]
    rem = vision_feature[rem_idx]
    cat = np.concatenate([sel, text_embed], axis=0)
    x = _ln_np(_mha_np(cat, cat, Wqkv1, bqkv1, Wo1, bo1) + cat)
    r = _ln_np(_mha_np(rem, rem, Wqkv2, bqkv2, Wo2, bo2) + rem)
    x = _ln_np(_mha_np(r, x, Wqkvc, bqkvc, Woc, boc) + r)
    ffn = _gelu_np(x @ Wf1.T + bf1) @ Wf2.T + bf2
    x = _ln_np(x + ffn)
    logits = (x @ Ws.T + bs).squeeze(-1)
    es = 1.0 / (1.0 + np.exp(-logits))
    k = int(t * EXPAND)
    ei = np.argsort(-es, kind="stable")[:k]
    final = np.sort(np.concatenate([sel_idx, rem_idx[ei]]))
    return vision_feature[final]


# ----------------------------------------------------------------------------
# device program
# ----------------------------------------------------------------------------

def _pad128(n):
    return ((n + 127) // 128) * 128


def _build_device(ncat_real, nrem_real, debug=False):
    import concourse.bacc as bacc
    import concourse.mybir as mybir
    import concourse.tile as tile

    dt = mybir.dt
    F32 = dt.float32
    F32R = dt.float32r
    F16 = dt.float16
    AF = mybir.ActivationFunctionType
    ALU = mybir.AluOpType

    ncat = _pad128(ncat_real)
    nrem = _pad128(nrem_real)

    nc = bacc.Bacc("TRN2", target_bir_lowering=False, debug=False,
                   num_devices=NCORES)

    # ---------------- DRAM I/O ----------------
    catT_d = nc.dram_tensor("catT", [E, ncat], F16, kind="ExternalInput")
    remT_d = nc.dram_tensor("remT", [E, nrem], F16, kind="ExternalInput")
    wd = {}
    for l in ("1", "2", "c"):
        for p in ("q", "k", "v"):
            wd[p + l] = nc.dram_tensor(f"w{p}{l}", [E, DLOC], F16,
                                       kind="ExternalInput")
        wd["o" + l] = nc.dram_tensor(f"wo{l}", [DLOC, E], F16,
                                     kind="ExternalInput")
    wd["f1"] = nc.dram_tensor("wf1", [E, FLOC], F16, kind="ExternalInput")
    wd["f2"] = nc.dram_tensor("wf2", [FLOC, E], F16, kind="ExternalInput")
    # row-sums of W_qc (over E) and Wf1_shard (over E), for the LN identity
    wqcr_d = nc.dram_tensor("wqcr", [1, DLOC], F16, kind="ExternalInput")
    wf1r_d = nc.dram_tensor("wf1r", [1, FLOC], F16, kind="ExternalInput")
    wsb_d = nc.dram_tensor("wsb", [128, 4], F16, kind="ExternalInput")
    masks_d = nc.dram_tensor("masks", [128, 4], F16, kind="ExternalInput")
    consts_d = nc.dram_tensor("consts", [1, 4], F32, kind="ExternalInput")
    logits_d = nc.dram_tensor("logits", [1, nrem], F32, kind="ExternalOutput")
    dbg = {}
    if debug:
        for nm, L in (("dbg_x1", ncat), ("dbg_r", nrem), ("dbg_x2", nrem)):
            dbg[nm] = nc.dram_tensor(nm, [E, L], F16, kind="ExternalOutput")

    replica = [list(range(NCORES))]

    with tile.TileContext(nc, num_cores=NCORES) as tc:
        with (
            tc.tile_pool(name="acts", bufs=1) as acts,
            tc.tile_pool(name="psum", bufs=1, space="PSUM") as psum,
            tc.tile_pool(name="dram", bufs=1, space="DRAM") as dram,
        ):
            # ---- constants / packed stat tiles ----
            ones_col = acts.tile([128, 1], F16, name="ones_col",
                                 tag="ones_col")
            nc.vector.memset(ones_col[:], 1.0)
            ones_row = acts.tile([1, 128], F32R, name="ones_row",
                                 tag="ones_row")
            nc.vector.memset(ones_row[:].bitcast(F32), 1.0)
            masks = acts.tile([128, 4], F16, name="masks", tag="masks")
            nc.sync.dma_start(masks[:], masks_d.ap())
            consts = acts.tile([1, 4], F32, name="consts", tag="consts")
            nc.sync.dma_start(consts[:], consts_d.ap())
            wqcr = acts.tile([1, DLOC], F16, name="wqcr", tag="wqcr")
            nc.sync.dma_start(wqcr[:], wqcr_d.ap())
            wf1r = acts.tile([1, FLOC], F16, name="wf1r", tag="wf1r")
            nc.sync.dma_start(wf1r[:], wf1r_d.ap())

            def pp(name, L):
                return psum.tile([128, L], F32, name=name, tag="pp", bufs=8)

            def pstat(name, L):
                return psum.tile([1, L], F32, name=name, tag="pp", bufs=8)

            def wtile(name, cols):
                return acts.tile([128, cols], F16, name=name, tag="wt",
                                 bufs=10, padded_shape=[128, 1024])

            def ar_bufs(tag, L, nchunks):
                rows = E // nchunks
                ins, outs = [], []
                for q in range(nchunks):
                    ins.append(dram.tile([rows, L], F16,
                                         name=f"arin{tag}_{q}",
                                         tag=f"arin{tag}_{q}"))
                    outs.append(dram.tile([rows, L], F16,
                                          name=f"arout{tag}_{q}",
                                          tag=f"arout{tag}_{q}",
                                          addr_space="Shared"))
                return ins, outs

            # ---------------- building blocks ----------------
            def load_xT(name, dram_t, L, tagbase):
                ts = []
                for k in range(ET):
                    xt = acts.tile([128, L], F16, name=f"{name}_{k}",
                                   tag=f"{tagbase}_{k}")
                    nc.sync.dma_start(xt[:],
                                      dram_t.ap()[128 * k:128 * (k + 1), :])
                    ts.append(xt)
                return ts

            def proj_fm(tagbase, w_dram, x_tiles, L, outtag):
                """q/k fm projection -> 4 tiles [128, L] (f16)."""
                ps = [pp(f"ps_{tagbase}_{m}", L) for m in range(4)]
                outs = []
                for k in range(ET):
                    wt = wtile(f"w_{tagbase}_{k}", DLOC)
                    nc.sync.dma_start(
                        wt[:], w_dram.ap()[128 * k:128 * (k + 1), :])
                    for m in range(4):
                        nc.tensor.matmul(ps[m][:],
                                         wt[:, 128 * m:128 * (m + 1)],
                                         x_tiles[k][:],
                                         start=(k == 0), stop=(k == ET - 1))
                for m in range(4):
                    o = acts.tile([128, L], F16, name=f"{tagbase}_{m}",
                                  tag=f"{outtag}_{m}")
                    nc.vector.tensor_copy(o[:], ps[m][:])
                    outs.append(o)
                return outs

            def proj_tm(tagbase, w_dram, x_tiles, L):
                """v tm projection -> L//128 tiles [128, DLOC] (f16)."""
                jt = L // 128
                ps = [pp(f"ps_{tagbase}_{j}", DLOC) for j in range(jt)]
                outs = []
                for k in range(ET):
                    wt = wtile(f"w_{tagbase}_{k}", DLOC)
                    nc.sync.dma_start(
                        wt[:], w_dram.ap()[128 * k:128 * (k + 1), :])
                    for j in range(jt):
                        nc.tensor.matmul(ps[j][:],
                                         x_tiles[k][:, 128 * j:128 * (j + 1)],
                                         wt[:],
                                         start=(k == 0), stop=(k == ET - 1))
                for j in range(jt):
                    o = acts.tile([128, DLOC], F16, name=f"{tagbase}_{j}",
                                  tag=f"v_{j}")
                    nc.vector.tensor_copy(o[:], ps[j][:])
                    outs.append(o)
                return outs

            def attention(tag, qT, kT, vT, Lq, Lkv, kv_valid, mask_idx):
                jt = Lkv // 128
                oT = []
                for h in range(HL):
                    exps = []
                    for j in range(jt):
                        p = pp(f"ps_s_{tag}_{h}_{j}", Lq)
                        for c in range(2):
                            nc.tensor.matmul(
                                p[:],
                                kT[2 * h + c][:, 128 * j:128 * (j + 1)],
                                qT[2 * h + c][:],
                                start=(c == 0), stop=(c == 1))
                        e = acts.tile([128, Lq], F16,
                                      name=f"es_{tag}_{h}_{j}",
                                      tag=f"expS_{j}")
                        nc.scalar.activation(e[:], p[:], AF.Exp,
                                             scale=float(1.0 / np.sqrt(DH)))
                        exps.append(e)
                    dsum = pstat(f"ps_d_{tag}_{h}", Lq)
                    for j in range(jt):
                        if j == jt - 1 and kv_valid < Lkv:
                            col = masks[:, mask_idx:mask_idx + 1]
                        else:
                            col = ones_col[:]
                        nc.tensor.matmul(dsum[:], col, exps[j][:],
                                         start=(j == 0), stop=(j == jt - 1))
                    den = acts.tile([1, Lq], F32, name=f"den_{tag}_{h}",
                                    tag="aden")
                    rec = acts.tile([1, Lq], F32, name=f"rec_{tag}_{h}",
                                    tag="arec")
                    nc.vector.tensor_copy(den[:], dsum[:])
                    nc.vector.reciprocal(rec[:], den[:])
                    nc.vector.tensor_tensor(den[:], den[:], rec[:], ALU.mult)
                    nc.vector.tensor_scalar(den[:], den[:], -1.0, 2.0,
                                            ALU.mult, ALU.add)
                    rec2 = acts.tile([1, Lq], F32R, name=f"rec2_{tag}_{h}",
                                     tag="rec2")
                    nc.vector.tensor_tensor(rec2[:], rec[:], den[:], ALU.mult)
                    rrep_p = pp(f"ps_rr_{tag}_{h}", Lq)
                    nc.tensor.matmul(rrep_p[:], ones_row[:], rec2[:],
                                     start=True, stop=True)
                    rrep = acts.tile([128, Lq], F32, name=f"rr_{tag}_{h}",
                                     tag="rrep")
                    nc.scalar.copy(rrep[:], rrep_p[:])
                    for c in range(2):
                        po = pp(f"ps_o_{tag}_{h}_{c}", Lq)
                        for j in range(jt):
                            nc.tensor.matmul(
                                po[:],
                                vT[j][:, 256 * h + 128 * c:
                                      256 * h + 128 * (c + 1)],
                                exps[j][:],
                                start=(j == 0), stop=(j == jt - 1))
                        o = acts.tile([128, Lq], F16,
                                      name=f"oT_{tag}_{h}_{c}",
                                      tag=f"oT_{2 * h + c}")
                        nc.vector.tensor_tensor(o[:], po[:], rrep[:],
                                                ALU.mult)
                        oT.append(o)
                return oT

            def out_proj_ar(tag, oT, w_dram, ar_ins, ar_outs, Lq):
                """Out-projection by feature quarter; each AllReduce chunk
                issued as soon as its quarters are staged."""
                nchunks = len(ar_ins)
                qpc = 4 // nchunks  # quarters per chunk
                for quarter in range(4):
                    ch, qi = quarter // qpc, quarter % qpc
                    wo_t = []
                    for k in range(4):
                        wt = wtile(f"wo_{tag}_{quarter}_{k}", 1024)
                        nc.sync.dma_start(
                            wt[:],
                            w_dram.ap()[128 * k:128 * (k + 1),
                                        1024 * quarter:1024 * (quarter + 1)])
                        wo_t.append(wt)
                    ps = [pp(f"ps_op_{tag}_{quarter}_{mm}", Lq)
                          for mm in range(8)]
                    for k in range(4):
                        for mm in range(8):
                            nc.tensor.matmul(
                                ps[mm][:],
                                wo_t[k][:, 128 * mm:128 * (mm + 1)],
                                oT[k][:],
                                start=(k == 0), stop=(k == 3))
                    for mm in range(8):
                        st = acts.tile([128, Lq], F16,
                                       name=f"st_{tag}_{quarter}_{mm}",
                                       tag="stage", bufs=6)
                        nc.vector.tensor_copy(st[:], ps[mm][:])
                        nc.sync.dma_start(
                            ar_ins[ch][1024 * qi + 128 * mm:
                                       1024 * qi + 128 * (mm + 1), :],
                            st[:])
                    if qi == qpc - 1:
                        nc.gpsimd.collective_compute(
                            "AllReduce", ALU.add, replica_groups=replica,
                            ins=[ar_ins[ch].opt()],
                            outs=[ar_outs[ch].opt()])

            def ln_stats_math(tag, s1p, s2p, L):
                """[1,L] LN stats from raw sum/sq-sum PSUMs.
                Returns (rstd F32R, nmr F32R, negmean16 F16)."""
                mean = acts.tile([1, L], F32, name=f"mean_{tag}",
                                 tag="lmean")
                var = acts.tile([1, L], F32, name=f"var_{tag}", tag="lvar")
                tmpa = acts.tile([1, L], F32, name=f"tmpa_{tag}", tag="ltmp")
                r0 = acts.tile([1, L], F32, name=f"r0_{tag}", tag="lr0")
                negmean16 = acts.tile([1, L], F16, name=f"nm16_{tag}",
                                      tag="nm16")
                nc.scalar.mul(mean[:], s1p[:], 1.0 / E)
                nc.scalar.mul(negmean16[:], s1p[:], -1.0 / E)
                nc.scalar.mul(var[:], s2p[:], 1.0 / E)
                nc.scalar.square(tmpa[:], mean[:])
                nc.vector.tensor_sub(var[:], var[:], tmpa[:])
                nc.vector.tensor_scalar_add(var[:], var[:], 1e-5)
                nc.scalar.sqrt(tmpa[:], var[:])
                nc.vector.reciprocal(r0[:], tmpa[:])
                nc.vector.tensor_tensor(tmpa[:], r0[:], r0[:], ALU.mult)
                nc.vector.tensor_tensor(tmpa[:], tmpa[:], var[:], ALU.mult)
                nc.vector.tensor_scalar(tmpa[:], tmpa[:], -0.5, 1.5, ALU.mult,
                                        ALU.add)
                rstd = acts.tile([1, L], F32R, name=f"rstd_{tag}", tag="rstd")
                nmr = acts.tile([1, L], F32R, name=f"nmr_{tag}", tag="nmr")
                nc.vector.tensor_tensor(rstd[:], r0[:], tmpa[:], ALU.mult)
                nc.vector.scalar_tensor_tensor(nmr[:], mean[:], -1.0, rstd[:],
                                               ALU.mult, ALU.mult)
                return rstd, nmr, negmean16

            def ln_broadcast(tag, rstd, nmr, L):
                """Broadcast [1,L] A=rstd, B=nmr to [128,L] via matmul."""
                Apsum = pp(f"ps_A_{tag}", L)
                nc.tensor.matmul(Apsum[:], ones_row[:], rstd[:], start=True,
                                 stop=True)
                Bpsum = pp(f"ps_B_{tag}", L)
                nc.tensor.matmul(Bpsum[:], ones_row[:], nmr[:], start=True,
                                 stop=True)
                Asb = acts.tile([128, L], F32, name=f"A_{tag}", tag="Asb")
                nc.scalar.copy(Asb[:], Apsum[:])
                Bsb = acts.tile([128, L], F32, name=f"B_{tag}", tag="Bsb")
                nc.scalar.copy(Bsb[:], Bpsum[:])
                return Asb, Bsb

            def residual_ln(tag, ar_outs, res_tiles, L, valid=None,
                            dump=None):
                """In-place: res_tiles[k] <- LN(sum_chunks(ar) + res)[k]."""
                nchunks = len(ar_outs)
                kc = ET // nchunks
                s1p = pstat(f"ps_s1_{tag}", L)
                s2p = pstat(f"ps_s2_{tag}", L)
                for k in range(ET):
                    q, mm = k // kc, k % kc
                    b = acts.tile([128, L], F16, name=f"arb_{tag}_{k}",
                                  tag="arb", bufs=8)
                    nc.sync.dma_start(
                        b[:], ar_outs[q][128 * mm:128 * (mm + 1), :])
                    nc.vector.tensor_tensor(res_tiles[k][:], b[:],
                                            res_tiles[k][:], ALU.add)
                    nc.tensor.matmul(s1p[:], ones_col[:], res_tiles[k][:],
                                     start=(k == 0), stop=(k == ET - 1))
                    sq = acts.tile([128, L], F16, name=f"sq_{tag}_{k}",
                                   tag="stage", bufs=6)
                    nc.scalar.square(sq[:], res_tiles[k][:])
                    nc.tensor.matmul(s2p[:], ones_col[:], sq[:],
                                     start=(k == 0), stop=(k == ET - 1))
                rstd, nmr, _ = ln_stats_math(tag, s1p, s2p, L)
                Asb, Bsb = ln_broadcast(tag, rstd, nmr, L)
                for k in range(ET):
                    nc.vector.tensor_tensor(res_tiles[k][:], res_tiles[k][:],
                                            Asb[:], ALU.mult)
                    nc.vector.tensor_tensor(res_tiles[k][:], res_tiles[k][:],
                                            Bsb[:], ALU.add)
                    if valid is not None and valid < L:
                        nc.vector.memset(res_tiles[k][:, valid:L], 0.0)
                    if dump is not None:
                        nc.sync.dma_start(
                            dump.ap()[128 * k:128 * (k + 1), :],
                            res_tiles[k][:])
                return res_tiles

            def ln_u_sweep(tag, w_dram, wcol0, nouts, wr_tile, ar_outs,
                           res_tiles, L, with_stats, stats=None):
                """Accumulate U[m] = W[:, wcol0+128m cols] @ y where
                y[k] = res[k] + ar chunk, consuming AR chunks as they land.
                If with_stats, also accumulate s1/s2 and finish the LN stats;
                then add the rank-1 rowsum(W) (x) (-mean) term so that
                U * rstd = W @ LN(y).  Returns (U psums, (rstd,nmr,Asb,Bsb)).
                res_tiles are left holding un-normalized y."""
                ps = [pp(f"ps_{tag}_{m}", L) for m in range(nouts)]
                if with_stats:
                    s1p = pstat(f"ps_s1_{tag}", L)
                    s2p = pstat(f"ps_s2_{tag}", L)
                for k in range(ET):
                    q, mm = k // KC, k % KC
                    if with_stats:
                        b = acts.tile([128, L], F16, name=f"arb_{tag}_{k}",
                                      tag="arb", bufs=8)
                        nc.sync.dma_start(
                            b[:], ar_outs[q][128 * mm:128 * (mm + 1), :])
                        nc.vector.tensor_tensor(res_tiles[k][:], b[:],
                                                res_tiles[k][:], ALU.add)
                        nc.tensor.matmul(s1p[:], ones_col[:],
                                         res_tiles[k][:],
                                         start=(k == 0), stop=(k == ET - 1))
                        sq = acts.tile([128, L], F16, name=f"sq_{tag}_{k}",
                                       tag="stage", bufs=6)
                        nc.scalar.square(sq[:], res_tiles[k][:])
                        nc.tensor.matmul(s2p[:], ones_col[:], sq[:],
                                         start=(k == 0), stop=(k == ET - 1))
                    wt = wtile(f"w_{tag}_{k}", 128 * nouts)
                    nc.sync.dma_start(
                        wt[:], w_dram.ap()[128 * k:128 * (k + 1),
                                           wcol0:wcol0 + 128 * nouts])
                    for m in range(nouts):
                        nc.tensor.matmul(ps[m][:],
                                         wt[:, 128 * m:128 * (m + 1)],
                                         res_tiles[k][:],
                                         start=(k == 0), stop=False)
                if with_stats:
                    rstd, nmr, negmean16 = ln_stats_math(tag, s1p, s2p, L)
                    Asb, Bsb = ln_broadcast(tag, rstd, nmr, L)
                    stats = (rstd, nmr, negmean16, Asb, Bsb)
                negmean16 = stats[2]
                for m in range(nouts):
                    nc.tensor.matmul(
                        ps[m][:],
                        wr_tile[0:1, wcol0 + 128 * m:wcol0 + 128 * (m + 1)],
                        negmean16[:],
                        start=False, stop=True)
                return ps, stats

            # ================= program =================
            # ---- MHA1 (cat self-attention) first; AR1 overlaps MHA2 ----
            c_t = load_xT("catT", catT_d, ncat, "b")
            q1 = proj_fm("q1", wd["q1"], c_t, ncat, "q")
            k1 = proj_fm("k1", wd["k1"], c_t, ncat, "k")
            v1 = proj_tm("v1", wd["v1"], c_t, ncat)
            o1 = attention("a1", q1, k1, v1, ncat, ncat, ncat_real, 0)
            arin1, arout1 = ar_bufs("1", ncat, 1)
            out_proj_ar("op1", o1, wd["o1"], arin1, arout1, ncat)

            # ---- MHA2 (rem self-attention), overlaps AR1 ----
            a_t = load_xT("remT", remT_d, nrem, "a")
            q2 = proj_fm("q2", wd["q2"], a_t, nrem, "q")
            k2 = proj_fm("k2", wd["k2"], a_t, nrem, "k")
            v2 = proj_tm("v2", wd["v2"], a_t, nrem)
            o2 = attention("a2", q2, k2, v2, nrem, nrem, nrem_real, 1)
            arin2, arout2 = ar_bufs("2", nrem, NCH)
            out_proj_ar("op2", o2, wd["o2"], arin2, arout2, nrem)

            # ---- x1 = LN(AR1 + cat); kc/vc fill the AR2 window ----
            x1_t = residual_ln("x1", arout1, c_t, ncat, valid=ncat_real,
                               dump=dbg.get("dbg_x1"))
            kc = proj_fm("kc", wd["kc"], x1_t, ncat, "k")
            vc = proj_tm("vc", wd["vc"], x1_t, ncat)

            # ---- qc = Wqc @ LN(AR2 + rem) via the LN identity,
            #      consuming AR2 chunk by chunk ----
            psq, st2 = ln_u_sweep("qc", wd["qc"], 0, 4, wqcr, arout2,
                                  a_t, nrem, True)
            A2sb, B2sb = st2[3], st2[4]
            qc = []
            for m in range(4):
                o = acts.tile([128, nrem], F16, name=f"qc_{m}", tag=f"q_{m}")
                nc.vector.tensor_tensor(o[:], psq[m][:], A2sb[:], ALU.mult)
                qc.append(o)
            # r = LN(y2) in place (residual for x2)
            for k in range(ET):
                nc.vector.tensor_tensor(a_t[k][:], a_t[k][:], A2sb[:],
                                        ALU.mult)
                nc.vector.tensor_tensor(a_t[k][:], a_t[k][:], B2sb[:],
                                        ALU.add)
                if dbg.get("dbg_r") is not None:
                    nc.sync.dma_start(
                        dbg["dbg_r"].ap()[128 * k:128 * (k + 1), :],
                        a_t[k][:])

            # ---- MHAc (q from r, kv from x1) ----
            oc = attention("ac", qc, kc, vc, nrem, ncat, ncat_real, 0)
            arinc, aroutc = ar_bufs("c", nrem, NCH)
            out_proj_ar("opc", oc, wd["oc"], arinc, aroutc, nrem)

            # ---- FFN f1 via the LN identity on y3 = ARc + r,
            #      consuming ARc chunk by chunk; two half-sweeps ----
            psfA, st3 = ln_u_sweep("f1A", wd["f1"], 0, 4, wf1r, aroutc,
                                   a_t, nrem, True)
            A3sb, B3sb = st3[3], st3[4]
            hT = []
            for m in range(4):
                hp = acts.tile([128, nrem], F16, name=f"hp_{m}",
                               tag="stage", bufs=6)
                nc.vector.tensor_tensor(hp[:], psfA[m][:], A3sb[:], ALU.mult)
                h = acts.tile([128, nrem], F16, name=f"hT_{m}", tag=f"v_{m}")
                nc.scalar.activation(h[:], hp[:], AF.Gelu)
                hT.append(h)
            psfB, _ = ln_u_sweep("f1B", wd["f1"], 512, 4, wf1r, aroutc,
                                 a_t, nrem, False, stats=st3)
            for m in range(4):
                hp = acts.tile([128, nrem], F16, name=f"hp_{m + 4}",
                               tag="stage", bufs=6)
                nc.vector.tensor_tensor(hp[:], psfB[m][:], A3sb[:], ALU.mult)
                h = acts.tile([128, nrem], F16, name=f"hT_{m + 4}",
                              tag=f"q_{m}")
                nc.scalar.activation(h[:], hp[:], AF.Gelu)
                hT.append(h)
            # x2 = LN(y3) in place (residual folded into FFN2 staging)
            for k in range(ET):
                nc.vector.tensor_tensor(a_t[k][:], a_t[k][:], A3sb[:],
                                        ALU.mult)
                nc.vector.tensor_tensor(a_t[k][:], a_t[k][:], B3sb[:],
                                        ALU.add)
                if dbg.get("dbg_x2") is not None:
                    nc.sync.dma_start(
                        dbg["dbg_x2"].ap()[128 * k:128 * (k + 1), :],
                        a_t[k][:])

            # ---- FFN f2 + fp16 ReduceScatter (2 chunks) ----
            HK = FLOC // 128  # 8
            rsin, rsout = [], []
            for ch in range(NCH):
                rsin.append(dram.tile([CROWS, nrem], F16, name=f"rsin_{ch}",
                                      tag=f"rsin_{ch}"))
                rsout.append(dram.tile([CROWS // NCORES, nrem], F16,
                                       name=f"rsout_{ch}",
                                       tag=f"rsout_{ch}"))
            for quarter in range(4):
                ch, qi = quarter // 2, quarter % 2
                ps = [pp(f"ps_f2_{quarter}_{mm}", nrem) for mm in range(8)]
                for khalf in range(2):
                    wf_t = []
                    for kk in range(4):
                        k = 4 * khalf + kk
                        wt = wtile(f"w_f2_{quarter}_{k}", 1024)
                        nc.sync.dma_start(
                            wt[:],
                            wd["f2"].ap()[128 * k:128 * (k + 1),
                                          1024 * quarter:1024 * (quarter + 1)])
                        wf_t.append(wt)
                    for kk in range(4):
                        k = 4 * khalf + kk
                        for mm in range(8):
                            nc.tensor.matmul(
                                ps[mm][:],
                                wf_t[kk][:, 128 * mm:128 * (mm + 1)],
                                hT[k][:],
                                start=(k == 0), stop=(k == HK - 1))
                for mm in range(8):
                    m = 8 * quarter + mm
                    st = acts.tile([128, nrem], F16, name=f"st_f2_{m}",
                                   tag="stage", bufs=6)
                    nc.vector.scalar_tensor_tensor(
                        st[:], a_t[m][:], 1.0 / NCORES, ps[mm][:],
                        ALU.mult, ALU.add)
                    nc.sync.dma_start(
                        rsin[ch][1024 * qi + 128 * mm:
                                 1024 * qi + 128 * (mm + 1), :], st[:])
                if qi == 1:
                    nc.gpsimd.collective_compute(
                        "ReduceScatter", ALU.add, replica_groups=replica,
                        ins=[rsin[ch].opt()], outs=[rsout[ch].opt()])

            # ---- final LN stats from scattered y = x2 + ffn ----
            wsb_sb = acts.tile([128, 4], F16, name="wsb_sb", tag="ws_sb")
            nc.sync.dma_start(wsb_sb[:], wsb_d.ap())
            s1p = pstat("ps_rs1", nrem)
            s2p = pstat("ps_rs2", nrem)
            wsp = pstat("ps_rsw", nrem)
            for ch in range(NCH):
                for j in range(CROWS // NCORES // 128):  # 2
                    idx = 2 * ch + j
                    bt = acts.tile([128, nrem], F16, name=f"rsb_{idx}",
                                   tag="arb", bufs=8)
                    nc.gpsimd.dma_start(bt[:],
                                        rsout[ch][128 * j:128 * (j + 1), :])
                    nc.tensor.matmul(s1p[:], ones_col[:], bt[:],
                                     start=(idx == 0), stop=(idx == 3))
                    nc.tensor.matmul(wsp[:], wsb_sb[:, idx:idx + 1], bt[:],
                                     start=(idx == 0), stop=(idx == 3))
                    sq = acts.tile([128, nrem], F16, name=f"rssq_{idx}",
                                   tag="stage", bufs=6)
                    nc.scalar.square(sq[:], bt[:])
                    nc.tensor.matmul(s2p[:], ones_col[:], sq[:],
                                     start=(idx == 0), stop=(idx == 3))
            s1s = acts.tile([1, nrem], F32, name="s1s", tag="lmean")
            s2s = acts.tile([1, nrem], F32, name="s2s", tag="lvar")
            wss = acts.tile([1, nrem], F32, name="wss", tag="lr0")
            nc.vector.tensor_copy(s1s[:], s1p[:])
            nc.vector.tensor_copy(s2s[:], s2p[:])
            nc.vector.tensor_copy(wss[:], wsp[:])
            arin5 = dram.tile([4, nrem], F32, name="arin5", tag="arin5")
            arout5 = dram.tile([4, nrem], F32, name="arout5",
                               tag="arout5", addr_space="Shared")
            nc.sync.dma_start(arin5[0:1, :], s1s[:])
            nc.sync.dma_start(arin5[1:2, :], s2s[:])
            nc.sync.dma_start(arin5[2:3, :], wss[:])
            nc.sync.dma_start(arin5[3:4, :], s1s[:])
            nc.gpsimd.collective_compute(
                "AllReduce", ALU.add, replica_groups=replica,
                ins=[arin5.opt()], outs=[arout5.opt()])
            g1 = acts.tile([1, nrem], F32, name="g1", tag="aden")
            g2 = acts.tile([1, nrem], F32, name="g2", tag="arec")
            g3 = acts.tile([1, nrem], F32, name="g3", tag="wsd")
            nc.sync.dma_start(g1[:], arout5[0:1, :])
            nc.sync.dma_start(g2[:], arout5[1:2, :])
            nc.sync.dma_start(g3[:], arout5[2:3, :])
            mean = acts.tile([1, nrem], F32, name="mean_l", tag="lmean")
            var = acts.tile([1, nrem], F32, name="var_l", tag="lvar")
            tmpa = acts.tile([1, nrem], F32, name="tmpa_l", tag="ltmp")
            r0 = acts.tile([1, nrem], F32, name="r0_l", tag="lr0")
            nc.scalar.mul(mean[:], g1[:], 1.0 / E)
            nc.scalar.mul(var[:], g2[:], 1.0 / E)
            nc.scalar.square(tmpa[:], mean[:])
            nc.vector.tensor_sub(var[:], var[:], tmpa[:])
            nc.vector.tensor_scalar_add(var[:], var[:], 1e-5)
            nc.scalar.sqrt(tmpa[:], var[:])
            nc.vector.reciprocal(r0[:], tmpa[:])
            nc.vector.tensor_tensor(tmpa[:], r0[:], r0[:], ALU.mult)
            nc.vector.tensor_tensor(tmpa[:], tmpa[:], var[:], ALU.mult)
            nc.vector.tensor_scalar(tmpa[:], tmpa[:], -0.5, 1.5,
                                    ALU.mult, ALU.add)
            rstd = acts.tile([1, nrem], F32, name="rstd_l", tag="rstd")
            nc.vector.tensor_tensor(rstd[:], r0[:], tmpa[:], ALU.mult)
            nmr = acts.tile([1, nrem], F32, name="nmr_l", tag="nmr")
            nc.vector.scalar_tensor_tensor(nmr[:], mean[:], -1.0,
                                           rstd[:], ALU.mult, ALU.mult)
            wdot = acts.tile([1, nrem], F32, name="wdot", tag="wdot")
            nc.vector.tensor_tensor(wdot[:], rstd[:], g3[:], ALU.mult)
            lsb = acts.tile([1, nrem], F32, name="lsb", tag="lsb")
            nc.vector.scalar_tensor_tensor(lsb[:], nmr[:],
                                           consts[0:1, 0:1], wdot[:],
                                           ALU.mult, ALU.add)
            nc.sync.dma_start(logits_d.ap(), lsb[:])

    nc.compile()
    return nc


# ----------------------------------------------------------------------------
# host orchestration
# ----------------------------------------------------------------------------

def _prep_in_maps(vision_feature, text_embed, sel_idx, rem_idx, ncat, nrem,
                  Wqkv1, Wo1, Wqkv2, Wo2, Wqkvc, Woc, Wf1, Wf2, Ws):
    f16 = np.float16
    sel = vision_feature[sel_idx]
    rem = vision_feature[rem_idx]
    cat = np.concatenate([sel, text_embed], axis=0)
    catT = np.zeros((E, ncat), f16)
    catT[:, :cat.shape[0]] = cat.T
    remT = np.zeros((E, nrem), f16)
    remT[:, :rem.shape[0]] = rem.T

    ncat_real = cat.shape[0]
    nrem_real = rem.shape[0]
    masks = np.zeros((128, 4), f16)
    masks[:ncat_real - 128 * (ncat // 128 - 1), 0] = 1.0
    masks[:nrem_real - 128 * (nrem // 128 - 1), 1] = 1.0
    consts = np.zeros((1, 4), np.float32)
    consts[0, 0] = Ws.astype(np.float64).sum()

    in_maps = []
    for c in range(NCORES):
        hs = slice(DLOC * c, DLOC * (c + 1))
        fs = slice(FLOC * c, FLOC * (c + 1))
        # core c's Ws rows for RS chunk ch, sub-tile j: [2048ch+256c+128j, +128)
        wsb = np.stack(
            [Ws[0, CROWS * ch + 256 * c + 128 * j:
                CROWS * ch + 256 * c + 128 * (j + 1)]
             for ch in range(NCH) for j in range(2)], axis=1).astype(f16)
        m = {"catT": catT, "remT": remT, "masks": masks, "consts": consts,
             "wsb": np.ascontiguousarray(wsb)}
        for l, Wqkv, Wo in (("1", Wqkv1, Wo1), ("2", Wqkv2, Wo2),
                            ("c", Wqkvc, Woc)):
            Wq, Wk, Wv = Wqkv[:E], Wqkv[E:2 * E], Wqkv[2 * E:]
            m["wq" + l] = np.ascontiguousarray(Wq[hs].T.astype(f16))
            m["wk" + l] = np.ascontiguousarray(Wk[hs].T.astype(f16))
            m["wv" + l] = np.ascontiguousarray(Wv[hs].T.astype(f16))
            m["wo" + l] = np.ascontiguousarray(Wo[:, hs].T.astype(f16))
        m["wf1"] = np.ascontiguousarray(Wf1[fs].T.astype(f16))
        m["wf2"] = np.ascontiguousarray(Wf2[:, fs].T.astype(f16))
        # row-sums over E for the LN-identity rank-1 terms (fp32 accum)
        m["wqcr"] = Wqkvc[:E][hs].sum(axis=1).astype(f16)[None, :]
        m["wf1r"] = Wf1[fs].sum(axis=1).astype(f16)[None, :]
        in_maps.append(m)
    return in_maps


def run_device(in_maps, ncat_real, nrem_real, debug=False, trace=False):
    from concourse.bass_utils import run_bass_kernel_spmd

    key = (ncat_real, nrem_real, debug)
    if key not in _CACHE:
        _CACHE[key] = _build_device(ncat_real, nrem_real, debug=debug)
    nc = _CACHE[key]
    return run_bass_kernel_spmd(nc, in_maps, list(range(NCORES)), trace=trace)


def _kernel_impl(inputs, debug=False, trace=False):
    vision_feature = np.asarray(inputs["vision_feature"], np.float32)
    text_embed = np.asarray(inputs["text_embed"], np.float32)
    attention_mask = np.asarray(inputs["attention_mask"])

    biases_zero = all(
        not np.any(np.asarray(inputs[b]))
        for b in ("bqkv1", "bo1", "bqkv2", "bo2", "bqkvc", "boc",
                  "bf1", "bf2", "bs"))
    if (not bool(attention_mask.all())) or (not biases_zero):
        return _reference_np(**{k: np.asarray(v) for k, v in inputs.items()}), None

    t, sel_idx, rem_idx = _score_partition(vision_feature, text_embed,
                                           attention_mask)
    ncat_real = t + text_embed.shape[0]
    nrem_real = vision_feature.shape[0] - t
    kk = int(t * EXPAND)

    in_maps = _prep_in_maps(
        vision_feature, text_embed, sel_idx, rem_idx,
        _pad128(ncat_real), _pad128(nrem_real),
        np.asarray(inputs["Wqkv1"], np.float32),
        np.asarray(inputs["Wo1"], np.float32),
        np.asarray(inputs["Wqkv2"], np.float32),
        np.asarray(inputs["Wo2"], np.float32),
        np.asarray(inputs["Wqkvc"], np.float32),
        np.asarray(inputs["Woc"], np.float32),
        np.asarray(inputs["Wf1"], np.float32),
        np.asarray(inputs["Wf2"], np.float32),
        np.asarray(inputs["Ws"], np.float32))
    res = run_device(in_maps, ncat_real, nrem_real, debug=debug, trace=trace)
    logits = res.results[0]["logits"][0, :nrem_real]
    es = (1.0 / (1.0 + np.exp(-logits.astype(np.float32))))
    ei = np.argsort(-es, kind="stable")[:kk]
    final = np.sort(np.concatenate([sel_idx, rem_idx[ei]]))
    return vision_feature[final], res


def kernel(**inputs):
    out, _ = _kernel_impl(inputs)
    return out


# revision 11
# speedup vs baseline: 1.1343x; 1.1343x over previous
"""Trainium2 Bass kernel for nn_CosSimRouter_learn_49778670960796.

Host: cosine-similarity scoring / sort / gather (tiny, shape-determining).
Device (8 NeuronCores, tensor-parallel over heads/hidden):
  3x MHA + FFN + logits; fp16 storage + matmuls (fp32 PSUM accum).
  Comm/compute overlap:
   - MHA1 (small) first; its AllReduce overlaps MHA2 (large).
   - AR2/ARc split into 2 feature-half chunks, issued as soon as their
     out-proj half is staged.
   - The consumers of AR2/ARc (cross-attn q-proj, FFN f1) use the
     affine-LN identity  W@LN(y) = (W@y + rowsum(W) (x) (-mean)) * rstd
     so their heavy weight sweeps consume un-normalized y chunk by chunk
     while the AllReduce is still in flight.
   - FFN2 output: fp16 ReduceScatter (2 chunks) + tiny stat AllReduce;
     logits from the affine-LN identity.
Host: top-k + final gather (exact rows of the input).
"""

import numpy as np

E = 4096
H = 16
HID = 8192
GAMMA = 0.2
TEMP = 0.05
EXPAND = 0.7
NCORES = 8
ET = E // 128  # 32 feature tiles
DH = E // H  # 256
HL = H // NCORES  # 2 heads per core
DLOC = HL * DH  # 512 local head dims
FLOC = HID // NCORES  # 1024 local ffn hidden
NCH = 2  # feature-half chunks per big collective
CROWS = E // NCH  # 2048
KC = ET // NCH  # 16 feature tiles per chunk

_CACHE = {}


# ----------------------------------------------------------------------------
# host-side reference math (numpy, fp32) for the scoring stage + fallback
# ----------------------------------------------------------------------------

def _score_partition(vision_feature, text_embed, attention_mask):
    vf = vision_feature.astype(np.float32)
    te = text_embed.astype(np.float32)
    vn = vf / np.maximum(np.linalg.norm(vf, axis=-1, keepdims=True), 1e-8)
    tn = te / np.maximum(np.linalg.norm(te, axis=-1, keepdims=True), 1e-8)
    cs = vn @ tn.T
    cs = np.where(attention_mask[None, :], cs, np.float32(0.0))
    m = cs.max(axis=-1) / np.float32(TEMP)
    e = np.exp(m - m.max())
    scores = e / e.sum()
    order = np.argsort(-scores, kind="stable")
    cum = np.cumsum(scores[order])
    t = int((cum <= GAMMA).sum())
    return t, order[:t], order[t:]


def _ln_np(x):
    m = x.mean(-1, keepdims=True)
    v = ((x - m) ** 2).mean(-1, keepdims=True)
    return (x - m) / np.sqrt(v + 1e-5)


def _gelu_np(x):
    import math

    erf = np.frompyfunc(math.erf, 1, 1)
    return (x * 0.5 * (1.0 + erf(x / math.sqrt(2.0)).astype(np.float64))
            ).astype(x.dtype)


def _mha_np(q_in, kv_in, Wqkv, bqkv, Wo, bo):
    dh = E // H
    Wq, Wk, Wv = np.split(Wqkv, 3, axis=0)
    bq, bk, bv = np.split(bqkv, 3)
    q = (q_in @ Wq.T + bq).reshape(-1, H, dh)
    k = (kv_in @ Wk.T + bk).reshape(-1, H, dh)
    v = (kv_in @ Wv.T + bv).reshape(-1, H, dh)
    att = np.einsum("qhd,khd->hqk", q, k) / np.float32(np.sqrt(dh))
    att = att - att.max(-1, keepdims=True)
    att = np.exp(att)
    att /= att.sum(-1, keepdims=True)
    o = np.einsum("hqk,khd->qhd", att.astype(np.float32), v).reshape(-1, E)
    return o @ Wo.T + bo


def _reference_np(vision_feature, text_embed, attention_mask,
                  Wqkv1, bqkv1, Wo1, bo1, Wqkv2, bqkv2, Wo2, bo2,
                  Wqkvc, bqkvc, Woc, boc, Wf1, bf1, Wf2, bf2, Ws, bs):
    t, sel_idx, rem_idx = _score_partition(vision_feature, text_embed,
                                           attention_mask)
    sel = vision_feature[sel_idx]
    rem = vision_feature[rem_idx]
    cat = np.concatenate([sel, text_embed], axis=0)
    x = _ln_np(_mha_np(cat, cat, Wqkv1, bqkv1, Wo1, bo1) + cat)
    r = _ln_np(_mha_np(rem, rem, Wqkv2, bqkv2, Wo2, bo2) + rem)
    x = _ln_np(_mha_np(r, x, Wqkvc, bqkvc, Woc, boc) + r)
    ffn = _gelu_np(x @ Wf1.T + bf1) @ Wf2.T + bf2
    x = _ln_np(x + ffn)
    logits = (x @ Ws.T + bs).squeeze(-1)
    es = 1.0 / (1.0 + np.exp(-logits))
    k = int(t * EXPAND)
    ei = np.argsort(-es, kind="stable")[:k]
    final = np.sort(np.concatenate([sel_idx, rem_idx[ei]]))
    return vision_feature[final]


# ----------------------------------------------------------------------------
# device program
# ----------------------------------------------------------------------------

def _pad128(n):
    return ((n + 127) // 128) * 128


def _build_device(ncat_real, nrem_real, debug=False):
    import concourse.bacc as bacc
    import concourse.mybir as mybir
    import concourse.tile as tile

    dt = mybir.dt
    F32 = dt.float32
    F32R = dt.float32r
    F16 = dt.float16
    AF = mybir.ActivationFunctionType
    ALU = mybir.AluOpType

    ncat = _pad128(ncat_real)
    nrem = _pad128(nrem_real)

    nc = bacc.Bacc("TRN2", target_bir_lowering=False, debug=False,
                   num_devices=NCORES)

    # ---------------- DRAM I/O ----------------
    catT_d = nc.dram_tensor("catT", [E, ncat], F16, kind="ExternalInput")
    remT_d = nc.dram_tensor("remT", [E, nrem], F16, kind="ExternalInput")
    wd = {}
    for l in ("1", "2", "c"):
        for p in ("q", "k", "v"):
            wd[p + l] = nc.dram_tensor(f"w{p}{l}", [E, DLOC], F16,
                                       kind="ExternalInput")
        wd["o" + l] = nc.dram_tensor(f"wo{l}", [DLOC, E], F16,
                                     kind="ExternalInput")
    wd["f1"] = nc.dram_tensor("wf1", [E, FLOC], F16, kind="ExternalInput")
    wd["f2"] = nc.dram_tensor("wf2", [FLOC, E], F16, kind="ExternalInput")
    # row-sums of W_qc (over E) and Wf1_shard (over E), for the LN identity
    wqcr_d = nc.dram_tensor("wqcr", [1, DLOC], F16, kind="ExternalInput")
    wf1r_d = nc.dram_tensor("wf1r", [1, FLOC], F16, kind="ExternalInput")
    wsb_d = nc.dram_tensor("wsb", [128, 4], F16, kind="ExternalInput")
    masks_d = nc.dram_tensor("masks", [128, 4], F16, kind="ExternalInput")
    consts_d = nc.dram_tensor("consts", [1, 4], F32, kind="ExternalInput")
    logits_d = nc.dram_tensor("logits", [1, nrem], F32, kind="ExternalOutput")
    dbg = {}
    if debug:
        for nm, L in (("dbg_x1", ncat), ("dbg_r", nrem), ("dbg_x2", nrem)):
            dbg[nm] = nc.dram_tensor(nm, [E, L], F16, kind="ExternalOutput")

    replica = [list(range(NCORES))]

    with tile.TileContext(nc, num_cores=NCORES) as tc:
        with (
            tc.tile_pool(name="acts", bufs=1) as acts,
            tc.tile_pool(name="psum", bufs=1, space="PSUM") as psum,
            tc.tile_pool(name="dram", bufs=1, space="DRAM") as dram,
        ):
            # ---- constants / packed stat tiles ----
            ones_col = acts.tile([128, 1], F16, name="ones_col",
                                 tag="ones_col")
            nc.vector.memset(ones_col[:], 1.0)
            ones_row = acts.tile([1, 128], F32R, name="ones_row",
                                 tag="ones_row")
            nc.vector.memset(ones_row[:].bitcast(F32), 1.0)
            masks = acts.tile([128, 4], F16, name="masks", tag="masks")
            nc.sync.dma_start(masks[:], masks_d.ap())
            consts = acts.tile([1, 4], F32, name="consts", tag="consts")
            nc.sync.dma_start(consts[:], consts_d.ap())
            wqcr = acts.tile([1, DLOC], F16, name="wqcr", tag="wqcr")
            nc.sync.dma_start(wqcr[:], wqcr_d.ap())
            wf1r = acts.tile([1, FLOC], F16, name="wf1r", tag="wf1r")
            nc.sync.dma_start(wf1r[:], wf1r_d.ap())

            def pp(name, L):
                return psum.tile([128, L], F32, name=name, tag="pp", bufs=8)

            def pstat(name, L):
                return psum.tile([1, L], F32, name=name, tag="pp", bufs=8)

            def wtile(name, cols):
                return acts.tile([128, cols], F16, name=name, tag="wt",
                                 bufs=10, padded_shape=[128, 1024])

            def ar_bufs(tag, L, nchunks):
                rows = E // nchunks
                ins, outs = [], []
                for q in range(nchunks):
                    ins.append(dram.tile([rows, L], F16,
                                         name=f"arin{tag}_{q}",
                                         tag=f"arin{tag}_{q}"))
                    outs.append(dram.tile([rows, L], F16,
                                          name=f"arout{tag}_{q}",
                                          tag=f"arout{tag}_{q}",
                                          addr_space="Shared"))
                return ins, outs

            # ---------------- building blocks ----------------
            def load_xT(name, dram_t, L, tagbase):
                ts = []
                for k in range(ET):
                    xt = acts.tile([128, L], F16, name=f"{name}_{k}",
                                   tag=f"{tagbase}_{k}")
                    nc.sync.dma_start(xt[:],
                                      dram_t.ap()[128 * k:128 * (k + 1), :])
                    ts.append(xt)
                return ts

            def proj_fm(tagbase, w_dram, x_tiles, L, outtag):
                """q/k fm projection -> 4 tiles [128, L] (f16)."""
                ps = [pp(f"ps_{tagbase}_{m}", L) for m in range(4)]
                outs = []
                for k in range(ET):
                    wt = wtile(f"w_{tagbase}_{k}", DLOC)
                    nc.sync.dma_start(
                        wt[:], w_dram.ap()[128 * k:128 * (k + 1), :])
                    for m in range(4):
                        nc.tensor.matmul(ps[m][:],
                                         wt[:, 128 * m:128 * (m + 1)],
                                         x_tiles[k][:],
                                         start=(k == 0), stop=(k == ET - 1))
                for m in range(4):
                    o = acts.tile([128, L], F16, name=f"{tagbase}_{m}",
                                  tag=f"{outtag}_{m}")
                    nc.scalar.copy(o[:], ps[m][:])
                    outs.append(o)
                return outs

            def proj_tm(tagbase, w_dram, x_tiles, L):
                """v tm projection -> L//128 tiles [128, DLOC] (f16)."""
                jt = L // 128
                ps = [pp(f"ps_{tagbase}_{j}", DLOC) for j in range(jt)]
                outs = []
                for k in range(ET):
                    wt = wtile(f"w_{tagbase}_{k}", DLOC)
                    nc.sync.dma_start(
                        wt[:], w_dram.ap()[128 * k:128 * (k + 1), :])
                    for j in range(jt):
                        nc.tensor.matmul(ps[j][:],
                                         x_tiles[k][:, 128 * j:128 * (j + 1)],
                                         wt[:],
                                         start=(k == 0), stop=(k == ET - 1))
                for j in range(jt):
                    o = acts.tile([128, DLOC], F16, name=f"{tagbase}_{j}",
                                  tag=f"v_{j}")
                    nc.scalar.copy(o[:], ps[j][:])
                    outs.append(o)
                return outs

            def attention(tag, qT, kT, vT, Lq, Lkv, kv_valid, mask_idx):
                jt = Lkv // 128
                oT = []
                for h in range(HL):
                    exps = []
                    for j in range(jt):
                        p = pp(f"ps_s_{tag}_{h}_{j}", Lq)
                        for c in range(2):
                            nc.tensor.matmul(
                                p[:],
                                kT[2 * h + c][:, 128 * j:128 * (j + 1)],
                                qT[2 * h + c][:],
                                start=(c == 0), stop=(c == 1))
                        e = acts.tile([128, Lq], F16,
                                      name=f"es_{tag}_{h}_{j}",
                                      tag=f"expS_{j}")
                        nc.scalar.activation(e[:], p[:], AF.Exp,
                                             scale=float(1.0 / np.sqrt(DH)))
                        exps.append(e)
                    dsum = pstat(f"ps_d_{tag}_{h}", Lq)
                    for j in range(jt):
                        if j == jt - 1 and kv_valid < Lkv:
                            col = masks[:, mask_idx:mask_idx + 1]
                        else:
                            col = ones_col[:]
                        nc.tensor.matmul(dsum[:], col, exps[j][:],
                                         start=(j == 0), stop=(j == jt - 1))
                    rec2 = acts.tile([1, Lq], F32R, name=f"rec2_{tag}_{h}",
                                     tag="rec2")
                    with nc.allow_low_precision(
                            reason="f32r output has f32 bits"):
                        nc.vector.reciprocal(rec2[:], dsum[:])
                    rrep_p = pp(f"ps_rr_{tag}_{h}", Lq)
                    nc.tensor.matmul(rrep_p[:], ones_row[:], rec2[:],
                                     start=True, stop=True)
                    rrep = acts.tile([128, Lq], F32, name=f"rr_{tag}_{h}",
                                     tag="rrep")
                    nc.scalar.copy(rrep[:], rrep_p[:])
                    for c in range(2):
                        po = pp(f"ps_o_{tag}_{h}_{c}", Lq)
                        for j in range(jt):
                            nc.tensor.matmul(
                                po[:],
                                vT[j][:, 256 * h + 128 * c:
                                      256 * h + 128 * (c + 1)],
                                exps[j][:],
                                start=(j == 0), stop=(j == jt - 1))
                        o = acts.tile([128, Lq], F16,
                                      name=f"oT_{tag}_{h}_{c}",
                                      tag=f"oT_{2 * h + c}")
                        nc.vector.tensor_tensor(o[:], po[:], rrep[:],
                                                ALU.mult)
                        oT.append(o)
                return oT

            def out_proj_ar(tag, oT, w_dram, ar_ins, ar_outs, Lq):
                """Out-projection by feature quarter; each AllReduce chunk
                issued as soon as its quarters are staged."""
                nchunks = len(ar_ins)
                qpc = 4 // nchunks  # quarters per chunk
                for quarter in range(4):
                    ch, qi = quarter // qpc, quarter % qpc
                    wo_t = []
                    for k in range(4):
                        wt = wtile(f"wo_{tag}_{quarter}_{k}", 1024)
                        nc.sync.dma_start(
                            wt[:],
                            w_dram.ap()[128 * k:128 * (k + 1),
                                        1024 * quarter:1024 * (quarter + 1)])
                        wo_t.append(wt)
                    ps = [pp(f"ps_op_{tag}_{quarter}_{mm}", Lq)
                          for mm in range(8)]
                    for k in range(4):
                        for mm in range(8):
                            nc.tensor.matmul(
                                ps[mm][:],
                                wo_t[k][:, 128 * mm:128 * (mm + 1)],
                                oT[k][:],
                                start=(k == 0), stop=(k == 3))
                    for mm in range(8):
                        st = acts.tile([128, Lq], F16,
                                       name=f"st_{tag}_{quarter}_{mm}",
                                       tag="stage", bufs=6)
                        nc.scalar.copy(st[:], ps[mm][:])
                        nc.sync.dma_start(
                            ar_ins[ch][1024 * qi + 128 * mm:
                                       1024 * qi + 128 * (mm + 1), :],
                            st[:])
                    if qi == qpc - 1:
                        nc.gpsimd.collective_compute(
                            "AllReduce", ALU.add, replica_groups=replica,
                            ins=[ar_ins[ch].opt()],
                            outs=[ar_outs[ch].opt()])

            def ln_stats_math(tag, s1p, s2p, L):
                """[1,L] LN stats from raw sum/sq-sum PSUMs.
                Returns (rstd F32R, nmr F32R, negmean16 F16)."""
                mean = acts.tile([1, L], F32, name=f"mean_{tag}",
                                 tag="lmean")
                var = acts.tile([1, L], F32, name=f"var_{tag}", tag="lvar")
                tmpa = acts.tile([1, L], F32, name=f"tmpa_{tag}", tag="ltmp")
                r0 = acts.tile([1, L], F32, name=f"r0_{tag}", tag="lr0")
                negmean16 = acts.tile([1, L], F16, name=f"nm16_{tag}",
                                      tag="nm16")
                nc.scalar.mul(mean[:], s1p[:], 1.0 / E)
                nc.scalar.mul(negmean16[:], s1p[:], -1.0 / E)
                nc.scalar.mul(var[:], s2p[:], 1.0 / E)
                nc.scalar.square(tmpa[:], mean[:])
                nc.vector.tensor_sub(var[:], var[:], tmpa[:])
                nc.vector.tensor_scalar_add(var[:], var[:], 1e-5)
                nc.scalar.sqrt(tmpa[:], var[:])
                nc.vector.reciprocal(r0[:], tmpa[:])
                nc.vector.tensor_tensor(tmpa[:], r0[:], r0[:], ALU.mult)
                nc.vector.tensor_tensor(tmpa[:], tmpa[:], var[:], ALU.mult)
                nc.vector.tensor_scalar(tmpa[:], tmpa[:], -0.5, 1.5, ALU.mult,
                                        ALU.add)
                rstd = acts.tile([1, L], F32R, name=f"rstd_{tag}", tag="rstd")
                nmr = acts.tile([1, L], F32R, name=f"nmr_{tag}", tag="nmr")
                nc.vector.tensor_tensor(rstd[:], r0[:], tmpa[:], ALU.mult)
                nc.vector.scalar_tensor_tensor(nmr[:], mean[:], -1.0, rstd[:],
                                               ALU.mult, ALU.mult)
                return rstd, nmr, negmean16

            def ln_broadcast(tag, rstd, nmr, L):
                """Broadcast [1,L] A=rstd, B=nmr to [128,L] via matmul."""
                Apsum = pp(f"ps_A_{tag}", L)
                nc.tensor.matmul(Apsum[:], ones_row[:], rstd[:], start=True,
                                 stop=True)
                Bpsum = pp(f"ps_B_{tag}", L)
                nc.tensor.matmul(Bpsum[:], ones_row[:], nmr[:], start=True,
                                 stop=True)
                Asb = acts.tile([128, L], F32, name=f"A_{tag}", tag="Asb")
                nc.scalar.copy(Asb[:], Apsum[:])
                Bsb = acts.tile([128, L], F32, name=f"B_{tag}", tag="Bsb")
                nc.scalar.copy(Bsb[:], Bpsum[:])
                return Asb, Bsb

            def residual_ln(tag, ar_outs, res_tiles, L, valid=None,
                            dump=None):
                """In-place: res_tiles[k] <- LN(sum_chunks(ar) + res)[k]."""
                nchunks = len(ar_outs)
                kc = ET // nchunks
                s1p = pstat(f"ps_s1_{tag}", L)
                s2p = pstat(f"ps_s2_{tag}", L)
                for k in range(ET):
                    q, mm = k // kc, k % kc
                    b = acts.tile([128, L], F16, name=f"arb_{tag}_{k}",
                                  tag="arb", bufs=8)
                    nc.sync.dma_start(
                        b[:], ar_outs[q][128 * mm:128 * (mm + 1), :])
                    nc.vector.tensor_tensor(res_tiles[k][:], b[:],
                                            res_tiles[k][:], ALU.add)
                    nc.tensor.matmul(s1p[:], ones_col[:], res_tiles[k][:],
                                     start=(k == 0), stop=(k == ET - 1))
                    sq = acts.tile([128, L], F16, name=f"sq_{tag}_{k}",
                                   tag="stage", bufs=6)
                    nc.scalar.square(sq[:], res_tiles[k][:])
                    nc.tensor.matmul(s2p[:], ones_col[:], sq[:],
                                     start=(k == 0), stop=(k == ET - 1))
                rstd, nmr, _ = ln_stats_math(tag, s1p, s2p, L)
                Asb, Bsb = ln_broadcast(tag, rstd, nmr, L)
                for k in range(ET):
                    nc.vector.tensor_tensor(res_tiles[k][:], res_tiles[k][:],
                                            Asb[:], ALU.mult)
                    nc.vector.tensor_tensor(res_tiles[k][:], res_tiles[k][:],
                                            Bsb[:], ALU.add)
                    if valid is not None and valid < L:
                        nc.vector.memset(res_tiles[k][:, valid:L], 0.0)
                    if dump is not None:
                        nc.sync.dma_start(
                            dump.ap()[128 * k:128 * (k + 1), :],
                            res_tiles[k][:])
                return res_tiles

            def ln_u_sweep(tag, w_dram, wcol0, nouts, wr_tile, ar_outs,
                           res_tiles, L, with_stats, stats=None):
                """Accumulate U[m] = W[:, wcol0+128m cols] @ y where
                y[k] = res[k] + ar chunk, consuming AR chunks as they land.
                If with_stats, also accumulate s1/s2 and finish the LN stats;
                then add the rank-1 rowsum(W) (x) (-mean) term so that
                U * rstd = W @ LN(y).  Returns (U psums, (rstd,nmr,Asb,Bsb)).
                res_tiles are left holding un-normalized y."""
                ps = [pp(f"ps_{tag}_{m}", L) for m in range(nouts)]
                if with_stats:
                    s1p = pstat(f"ps_s1_{tag}", L)
                    s2p = pstat(f"ps_s2_{tag}", L)
                for k in range(ET):
                    q, mm = k // KC, k % KC
                    if with_stats:
                        b = acts.tile([128, L], F16, name=f"arb_{tag}_{k}",
                                      tag="arb", bufs=8)
                        nc.sync.dma_start(
                            b[:], ar_outs[q][128 * mm:128 * (mm + 1), :])
                        nc.vector.tensor_tensor(res_tiles[k][:], b[:],
                                                res_tiles[k][:], ALU.add)
                        nc.tensor.matmul(s1p[:], ones_col[:],
                                         res_tiles[k][:],
                                         start=(k == 0), stop=(k == ET - 1))
                        sq = acts.tile([128, L], F16, name=f"sq_{tag}_{k}",
                                       tag="stage", bufs=6)
                        nc.scalar.square(sq[:], res_tiles[k][:])
                        nc.tensor.matmul(s2p[:], ones_col[:], sq[:],
                                         start=(k == 0), stop=(k == ET - 1))
                    wt = wtile(f"w_{tag}_{k}", 128 * nouts)
                    nc.sync.dma_start(
                        wt[:], w_dram.ap()[128 * k:128 * (k + 1),
                                           wcol0:wcol0 + 128 * nouts])
                    for m in range(nouts):
                        nc.tensor.matmul(ps[m][:],
                                         wt[:, 128 * m:128 * (m + 1)],
                                         res_tiles[k][:],
                                         start=(k == 0), stop=False)
                if with_stats:
                    rstd, nmr, negmean16 = ln_stats_math(tag, s1p, s2p, L)
                    Asb, Bsb = ln_broadcast(tag, rstd, nmr, L)
                    stats = (rstd, nmr, negmean16, Asb, Bsb)
                negmean16 = stats[2]
                for m in range(nouts):
                    nc.tensor.matmul(
                        ps[m][:],
                        wr_tile[0:1, wcol0 + 128 * m:wcol0 + 128 * (m + 1)],
                        negmean16[:],
                        start=False, stop=True)
                return ps, stats

            # ================= program =================
            # ---- MHA1 (cat self-attention) first; AR1 overlaps MHA2 ----
            c_t = load_xT("catT", catT_d, ncat, "b")
            q1 = proj_fm("q1", wd["q1"], c_t, ncat, "q")
            k1 = proj_fm("k1", wd["k1"], c_t, ncat, "k")
            v1 = proj_tm("v1", wd["v1"], c_t, ncat)
            a_t = load_xT("remT", remT_d, nrem, "a")
            o1 = attention("a1", q1, k1, v1, ncat, ncat, ncat_real, 0)
            # MHA2 projections issued before op1 so the tensor engine has
            # work while a1's softmax chain runs on vector/scalar
            q2 = proj_fm("q2", wd["q2"], a_t, nrem, "q")
            k2 = proj_fm("k2", wd["k2"], a_t, nrem, "k")
            v2 = proj_tm("v2", wd["v2"], a_t, nrem)
            arin1, arout1 = ar_bufs("1", ncat, 1)
            out_proj_ar("op1", o1, wd["o1"], arin1, arout1, ncat)
            o2 = attention("a2", q2, k2, v2, nrem, nrem, nrem_real, 1)
            arin2, arout2 = ar_bufs("2", nrem, NCH)
            out_proj_ar("op2", o2, wd["o2"], arin2, arout2, nrem)

            # ---- x1 = LN(AR1 + cat); kc/vc fill the AR2 window ----
            x1_t = residual_ln("x1", arout1, c_t, ncat, valid=ncat_real,
                               dump=dbg.get("dbg_x1"))
            kc = proj_fm("kc", wd["kc"], x1_t, ncat, "k")
            vc = proj_tm("vc", wd["vc"], x1_t, ncat)

            # ---- qc = Wqc @ LN(AR2 + rem) via the LN identity,
            #      consuming AR2 chunk by chunk ----
            psq, st2 = ln_u_sweep("qc", wd["qc"], 0, 4, wqcr, arout2,
                                  a_t, nrem, True)
            A2sb, B2sb = st2[3], st2[4]
            qc = []
            for m in range(4):
                o = acts.tile([128, nrem], F16, name=f"qc_{m}", tag=f"q_{m}")
                nc.vector.tensor_tensor(o[:], psq[m][:], A2sb[:], ALU.mult)
                qc.append(o)

            # ---- MHAc (q from r, kv from x1) ----
            oc = attention("ac", qc, kc, vc, nrem, ncat, ncat_real, 0)
            arinc, aroutc = ar_bufs("c", nrem, NCH)
            out_proj_ar("opc", oc, wd["oc"], arinc, aroutc, nrem)
            # r = LN(y2) in place (residual for x2); issued after opc so
            # these 64 vector ops overlap the ARc chunks instead of
            # blocking attention-ac's vector work
            for k in range(ET):
                nc.vector.tensor_tensor(a_t[k][:], a_t[k][:], A2sb[:],
                                        ALU.mult)
                nc.vector.tensor_tensor(a_t[k][:], a_t[k][:], B2sb[:],
                                        ALU.add)
                if dbg.get("dbg_r") is not None:
                    nc.sync.dma_start(
                        dbg["dbg_r"].ap()[128 * k:128 * (k + 1), :],
                        a_t[k][:])

            # ---- FFN f1 via the LN identity on y3 = ARc + r,
            #      consuming ARc chunk by chunk; two half-sweeps ----
            psfA, st3 = ln_u_sweep("f1A", wd["f1"], 0, 4, wf1r, aroutc,
                                   a_t, nrem, True)
            A3sb, B3sb = st3[3], st3[4]
            hT = []
            for m in range(4):
                hp = acts.tile([128, nrem], F16, name=f"hp_{m}",
                               tag="stage", bufs=6)
                nc.vector.tensor_tensor(hp[:], psfA[m][:], A3sb[:], ALU.mult)
                h = acts.tile([128, nrem], F16, name=f"hT_{m}", tag=f"v_{m}")
                nc.scalar.activation(h[:], hp[:], AF.Gelu)
                hT.append(h)
            psfB, _ = ln_u_sweep("f1B", wd["f1"], 512, 4, wf1r, aroutc,
                                 a_t, nrem, False, stats=st3)
            for m in range(4):
                hp = acts.tile([128, nrem], F16, name=f"hp_{m + 4}",
                               tag="stage", bufs=6)
                nc.vector.tensor_tensor(hp[:], psfB[m][:], A3sb[:], ALU.mult)
                h = acts.tile([128, nrem], F16, name=f"hT_{m + 4}",
                              tag=f"q_{m}")
                nc.scalar.activation(h[:], hp[:], AF.Gelu)
                hT.append(h)
            # x2 = LN(y3) in place (residual folded into FFN2 staging)
            for k in range(ET):
                nc.vector.tensor_tensor(a_t[k][:], a_t[k][:], A3sb[:],
                                        ALU.mult)
                nc.vector.tensor_tensor(a_t[k][:], a_t[k][:], B3sb[:],
                                        ALU.add)
                if dbg.get("dbg_x2") is not None:
                    nc.sync.dma_start(
                        dbg["dbg_x2"].ap()[128 * k:128 * (k + 1), :],
                        a_t[k][:])

            # ---- FFN f2 + fp16 ReduceScatter (2 chunks) ----
            HK = FLOC // 128  # 8
            rsin, rsout = [], []
            for ch in range(NCH):
                rsin.append(dram.tile([CROWS, nrem], F16, name=f"rsin_{ch}",
                                      tag=f"rsin_{ch}"))
                rsout.append(dram.tile([CROWS // NCORES, nrem], F16,
                                       name=f"rsout_{ch}",
                                       tag=f"rsout_{ch}"))
            for quarter in range(4):
                ch, qi = quarter // 2, quarter % 2
                ps = [pp(f"ps_f2_{quarter}_{mm}", nrem) for mm in range(8)]
                for khalf in range(2):
                    wf_t = []
                    for kk in range(4):
                        k = 4 * khalf + kk
                        wt = wtile(f"w_f2_{quarter}_{k}", 1024)
                        nc.sync.dma_start(
                            wt[:],
                            wd["f2"].ap()[128 * k:128 * (k + 1),
                                          1024 * quarter:1024 * (quarter + 1)])
                        wf_t.append(wt)
                    for kk in range(4):
                        k = 4 * khalf + kk
                        for mm in range(8):
                            nc.tensor.matmul(
                                ps[mm][:],
                                wf_t[kk][:, 128 * mm:128 * (mm + 1)],
                                hT[k][:],
                                start=(k == 0), stop=(k == HK - 1))
                for mm in range(8):
                    m = 8 * quarter + mm
                    st = acts.tile([128, nrem], F16, name=f"st_f2_{m}",
                                   tag="stage", bufs=6)
                    nc.vector.scalar_tensor_tensor(
                        st[:], a_t[m][:], 1.0 / NCORES, ps[mm][:],
                        ALU.mult, ALU.add)
                    nc.sync.dma_start(
                        rsin[ch][1024 * qi + 128 * mm:
                                 1024 * qi + 128 * (mm + 1), :], st[:])
                if qi == 1:
                    nc.gpsimd.collective_compute(
                        "ReduceScatter", ALU.add, replica_groups=replica,
                        ins=[rsin[ch].opt()], outs=[rsout[ch].opt()])

            # ---- final LN stats from scattered y = x2 + ffn ----
            wsb_sb = acts.tile([128, 4], F16, name="wsb_sb", tag="ws_sb")
            nc.sync.dma_start(wsb_sb[:], wsb_d.ap())
            s1p = pstat("ps_rs1", nrem)
            s2p = pstat("ps_rs2", nrem)
            wsp = pstat("ps_rsw", nrem)
            for ch in range(NCH):
                for j in range(CROWS // NCORES // 128):  # 2
                    idx = 2 * ch + j
                    bt = acts.tile([128, nrem], F16, name=f"rsb_{idx}",
                                   tag="arb", bufs=8)
                    nc.gpsimd.dma_start(bt[:],
                                        rsout[ch][128 * j:128 * (j + 1), :])
                    nc.tensor.matmul(s1p[:], ones_col[:], bt[:],
                                     start=(idx == 0), stop=(idx == 3))
                    nc.tensor.matmul(wsp[:], wsb_sb[:, idx:idx + 1], bt[:],
                                     start=(idx == 0), stop=(idx == 3))
                    sq = acts.tile([128, nrem], F16, name=f"rssq_{idx}",
                                   tag="stage", bufs=6)
                    nc.scalar.square(sq[:], bt[:])
                    nc.tensor.matmul(s2p[:], ones_col[:], sq[:],
                                     start=(idx == 0), stop=(idx == 3))
            s1s = acts.tile([1, nrem], F32, name="s1s", tag="lmean")
            s2s = acts.tile([1, nrem], F32, name="s2s", tag="lvar")
            wss = acts.tile([1, nrem], F32, name="wss", tag="lr0")
            nc.vector.tensor_copy(s1s[:], s1p[:])
            nc.vector.tensor_copy(s2s[:], s2p[:])
            nc.vector.tensor_copy(wss[:], wsp[:])
            arin5 = dram.tile([4, nrem], F32, name="arin5", tag="arin5")
            arout5 = dram.tile([4, nrem], F32, name="arout5",
                               tag="arout5", addr_space="Shared")
            nc.sync.dma_start(arin5[0:1, :], s1s[:])
            nc.sync.dma_start(arin5[1:2, :], s2s[:])
            nc.sync.dma_start(arin5[2:3, :], wss[:])
            nc.sync.dma_start(arin5[3:4, :], s1s[:])
            nc.gpsimd.collective_compute(
                "AllReduce", ALU.add, replica_groups=replica,
                ins=[arin5.opt()], outs=[arout5.opt()])
            g1 = acts.tile([1, nrem], F32, name="g1", tag="aden")
            g2 = acts.tile([1, nrem], F32, name="g2", tag="arec")
            g3 = acts.tile([1, nrem], F32, name="g3", tag="wsd")
            nc.sync.dma_start(g1[:], arout5[0:1, :])
            nc.sync.dma_start(g2[:], arout5[1:2, :])
            nc.sync.dma_start(g3[:], arout5[2:3, :])
            mean = acts.tile([1, nrem], F32, name="mean_l", tag="lmean")
            var = acts.tile([1, nrem], F32, name="var_l", tag="lvar")
            tmpa = acts.tile([1, nrem], F32, name="tmpa_l", tag="ltmp")
            r0 = acts.tile([1, nrem], F32, name="r0_l", tag="lr0")
            nc.scalar.mul(mean[:], g1[:], 1.0 / E)
            nc.scalar.mul(var[:], g2[:], 1.0 / E)
            nc.scalar.square(tmpa[:], mean[:])
            nc.vector.tensor_sub(var[:], var[:], tmpa[:])
            nc.vector.tensor_scalar_add(var[:], var[:], 1e-5)
            nc.scalar.sqrt(tmpa[:], var[:])
            nc.vector.reciprocal(r0[:], tmpa[:])
            nc.vector.tensor_tensor(tmpa[:], r0[:], r0[:], ALU.mult)
            nc.vector.tensor_tensor(tmpa[:], tmpa[:], var[:], ALU.mult)
            nc.vector.tensor_scalar(tmpa[:], tmpa[:], -0.5, 1.5,
                                    ALU.mult, ALU.add)
            rstd = acts.tile([1, nrem], F32, name="rstd_l", tag="rstd")
            nc.vector.tensor_tensor(rstd[:], r0[:], tmpa[:], ALU.mult)
            nmr = acts.tile([1, nrem], F32, name="nmr_l", tag="nmr")
            nc.vector.scalar_tensor_tensor(nmr[:], mean[:], -1.0,
                                           rstd[:], ALU.mult, ALU.mult)
            wdot = acts.tile([1, nrem], F32, name="wdot", tag="wdot")
            nc.vector.tensor_tensor(wdot[:], rstd[:], g3[:], ALU.mult)
            lsb = acts.tile([1, nrem], F32, name="lsb", tag="lsb")
            nc.vector.scalar_tensor_tensor(lsb[:], nmr[:],
                                           consts[0:1, 0:1], wdot[:],
                                           ALU.mult, ALU.add)
            nc.sync.dma_start(logits_d.ap(), lsb[:])

    nc.compile()
    return nc


# ----------------------------------------------------------------------------
# host orchestration
# ----------------------------------------------------------------------------

def _prep_in_maps(vision_feature, text_embed, sel_idx, rem_idx, ncat, nrem,
                  Wqkv1, Wo1, Wqkv2, Wo2, Wqkvc, Woc, Wf1, Wf2, Ws):
    f16 = np.float16
    sel = vision_feature[sel_idx]
    rem = vision_feature[rem_idx]
    cat = np.concatenate([sel, text_embed], axis=0)
    catT = np.zeros((E, ncat), f16)
    catT[:, :cat.shape[0]] = cat.T
    remT = np.zeros((E, nrem), f16)
    remT[:, :rem.shape[0]] = rem.T

    ncat_real = cat.shape[0]
    nrem_real = rem.shape[0]
    masks = np.zeros((128, 4), f16)
    masks[:ncat_real - 128 * (ncat // 128 - 1), 0] = 1.0
    masks[:nrem_real - 128 * (nrem // 128 - 1), 1] = 1.0
    consts = np.zeros((1, 4), np.float32)
    consts[0, 0] = Ws.astype(np.float64).sum()

    in_maps = []
    for c in range(NCORES):
        hs = slice(DLOC * c, DLOC * (c + 1))
        fs = slice(FLOC * c, FLOC * (c + 1))
        # core c's Ws rows for RS chunk ch, sub-tile j: [2048ch+256c+128j, +128)
        wsb = np.stack(
            [Ws[0, CROWS * ch + 256 * c + 128 * j:
                CROWS * ch + 256 * c + 128 * (j + 1)]
             for ch in range(NCH) for j in range(2)], axis=1).astype(f16)
        m = {"catT": catT, "remT": remT, "masks": masks, "consts": consts,
             "wsb": np.ascontiguousarray(wsb)}
        for l, Wqkv, Wo in (("1", Wqkv1, Wo1), ("2", Wqkv2, Wo2),
                            ("c", Wqkvc, Woc)):
            Wq, Wk, Wv = Wqkv[:E], Wqkv[E:2 * E], Wqkv[2 * E:]
            m["wq" + l] = np.ascontiguousarray(Wq[hs].T.astype(f16))
            m["wk" + l] = np.ascontiguousarray(Wk[hs].T.astype(f16))
            m["wv" + l] = np.ascontiguousarray(Wv[hs].T.astype(f16))
            m["wo" + l] = np.ascontiguousarray(Wo[:, hs].T.astype(f16))
        m["wf1"] = np.ascontiguousarray(Wf1[fs].T.astype(f16))
        m["wf2"] = np.ascontiguousarray(Wf2[:, fs].T.astype(f16))
        # row-sums over E for the LN-identity rank-1 terms (fp32 accum)
        m["wqcr"] = Wqkvc[:E][hs].sum(axis=1).astype(f16)[None, :]
        m["wf1r"] = Wf1[fs].sum(axis=1).astype(f16)[None, :]
        in_maps.append(m)
    return in_maps


def run_device(in_maps, ncat_real, nrem_real, debug=False, trace=False):
    from concourse.bass_utils import run_bass_kernel_spmd

    key = (ncat_real, nrem_real, debug)
    if key not in _CACHE:
        _CACHE[key] = _build_device(ncat_real, nrem_real, debug=debug)
    nc = _CACHE[key]
    return run_bass_kernel_spmd(nc, in_maps, list(range(NCORES)), trace=trace)


def _kernel_impl(inputs, debug=False, trace=False):
    vision_feature = np.asarray(inputs["vision_feature"], np.float32)
    text_embed = np.asarray(inputs["text_embed"], np.float32)
    attention_mask = np.asarray(inputs["attention_mask"])

    biases_zero = all(
        not np.any(np.asarray(inputs[b]))
        for b in ("bqkv1", "bo1", "bqkv2", "bo2", "bqkvc", "boc",
                  "bf1", "bf2", "bs"))
    if (not bool(attention_mask.all())) or (not biases_zero):
        return _reference_np(**{k: np.asarray(v) for k, v in inputs.items()}), None

    t, sel_idx, rem_idx = _score_partition(vision_feature, text_embed,
                                           attention_mask)
    ncat_real = t + text_embed.shape[0]
    nrem_real = vision_feature.shape[0] - t
    kk = int(t * EXPAND)

    in_maps = _prep_in_maps(
        vision_feature, text_embed, sel_idx, rem_idx,
        _pad128(ncat_real), _pad128(nrem_real),
        np.asarray(inputs["Wqkv1"], np.float32),
        np.asarray(inputs["Wo1"], np.float32),
        np.asarray(inputs["Wqkv2"], np.float32),
        np.asarray(inputs["Wo2"], np.float32),
        np.asarray(inputs["Wqkvc"], np.float32),
        np.asarray(inputs["Woc"], np.float32),
        np.asarray(inputs["Wf1"], np.float32),
        np.asarray(inputs["Wf2"], np.float32),
        np.asarray(inputs["Ws"], np.float32))
    res = run_device(in_maps, ncat_real, nrem_real, debug=debug, trace=trace)
    logits = res.results[0]["logits"][0, :nrem_real]
    es = (1.0 / (1.0 + np.exp(-logits.astype(np.float32))))
    ei = np.argsort(-es, kind="stable")[:kk]
    final = np.sort(np.concatenate([sel_idx, rem_idx[ei]]))
    return vision_feature[final], res


def kernel(**inputs):
    out, _ = _kernel_impl(inputs)
    return out


# revision 24
# speedup vs baseline: 1.1680x; 1.0297x over previous
"""Trainium2 Bass kernel for nn_CosSimRouter_learn_49778670960796.

Host: cosine-similarity scoring / sort / gather (tiny, shape-determining).
Device (8 NeuronCores, tensor-parallel over heads/hidden):
  3x MHA + FFN + logits; fp16 storage + matmuls (fp32 PSUM accum).
  Comm/compute overlap:
   - MHA1 (small) first; its AllReduce overlaps MHA2 (large).
   - AR2/ARc split into 2 feature-half chunks, issued as soon as their
     out-proj half is staged.
   - The consumers of AR2/ARc (cross-attn q-proj, FFN f1) use the
     affine-LN identity  W@LN(y) = (W@y + rowsum(W) (x) (-mean)) * rstd
     so their heavy weight sweeps consume un-normalized y chunk by chunk
     while the AllReduce is still in flight.
   - FFN2 output: fp16 ReduceScatter (2 chunks) + tiny stat AllReduce;
     logits from the affine-LN identity.
Host: top-k + final gather (exact rows of the input).
"""

import numpy as np

E = 4096
H = 16
HID = 8192
GAMMA = 0.2
TEMP = 0.05
EXPAND = 0.7
NCORES = 8
ET = E // 128  # 32 feature tiles
DH = E // H  # 256
HL = H // NCORES  # 2 heads per core
DLOC = HL * DH  # 512 local head dims
FLOC = HID // NCORES  # 1024 local ffn hidden
NCH = 2  # feature-half chunks per big collective
CROWS = E // NCH  # 2048
KC = ET // NCH  # 16 feature tiles per chunk

_CACHE = {}


# ----------------------------------------------------------------------------
# host-side reference math (numpy, fp32) for the scoring stage + fallback
# ----------------------------------------------------------------------------

def _score_partition(vision_feature, text_embed, attention_mask):
    vf = vision_feature.astype(np.float32)
    te = text_embed.astype(np.float32)
    vn = vf / np.maximum(np.linalg.norm(vf, axis=-1, keepdims=True), 1e-8)
    tn = te / np.maximum(np.linalg.norm(te, axis=-1, keepdims=True), 1e-8)
    cs = vn @ tn.T
    cs = np.where(attention_mask[None, :], cs, np.float32(0.0))
    m = cs.max(axis=-1) / np.float32(TEMP)
    e = np.exp(m - m.max())
    scores = e / e.sum()
    order = np.argsort(-scores, kind="stable")
    cum = np.cumsum(scores[order])
    t = int((cum <= GAMMA).sum())
    return t, order[:t], order[t:]


def _ln_np(x):
    m = x.mean(-1, keepdims=True)
    v = ((x - m) ** 2).mean(-1, keepdims=True)
    return (x - m) / np.sqrt(v + 1e-5)


def _gelu_np(x):
    import math

    erf = np.frompyfunc(math.erf, 1, 1)
    return (x * 0.5 * (1.0 + erf(x / math.sqrt(2.0)).astype(np.float64))
            ).astype(x.dtype)


def _mha_np(q_in, kv_in, Wqkv, bqkv, Wo, bo):
    dh = E // H
    Wq, Wk, Wv = np.split(Wqkv, 3, axis=0)
    bq, bk, bv = np.split(bqkv, 3)
    q = (q_in @ Wq.T + bq).reshape(-1, H, dh)
    k = (kv_in @ Wk.T + bk).reshape(-1, H, dh)
    v = (kv_in @ Wv.T + bv).reshape(-1, H, dh)
    att = np.einsum("qhd,khd->hqk", q, k) / np.float32(np.sqrt(dh))
    att = att - att.max(-1, keepdims=True)
    att = np.exp(att)
    att /= att.sum(-1, keepdims=True)
    o = np.einsum("hqk,khd->qhd", att.astype(np.float32), v).reshape(-1, E)
    return o @ Wo.T + bo


def _reference_np(vision_feature, text_embed, attention_mask,
                  Wqkv1, bqkv1, Wo1, bo1, Wqkv2, bqkv2, Wo2, bo2,
                  Wqkvc, bqkvc, Woc, boc, Wf1, bf1, Wf2, bf2, Ws, bs):
    t, sel_idx, rem_idx = _score_partition(vision_feature, text_embed,
                                           attention_mask)
    sel = vision_feature[sel_idx]
    rem = vision_feature[rem_idx]
    cat = np.concatenate([sel, text_embed], axis=0)
    x = _ln_np(_mha_np(cat, cat, Wqkv1, bqkv1, Wo1, bo1) + cat)
    r = _ln_np(_mha_np(rem, rem, Wqkv2, bqkv2, Wo2, bo2) + rem)
    x = _ln_np(_mha_np(r, x, Wqkvc, bqkvc, Woc, boc) + r)
    ffn = _gelu_np(x @ Wf1.T + bf1) @ Wf2.T + bf2
    x = _ln_np(x + ffn)
    logits = (x @ Ws.T + bs).squeeze(-1)
    es = 1.0 / (1.0 + np.exp(-logits))
    k = int(t * EXPAND)
    ei = np.argsort(-es, kind="stable")[:k]
    final = np.sort(np.concatenate([sel_idx, rem_idx[ei]]))
    return vision_feature[final]


# ----------------------------------------------------------------------------
# device program
# ----------------------------------------------------------------------------

def _pad128(n):
    return ((n + 127) // 128) * 128


def _build_device(ncat_real, nrem_real, debug=False):
    import concourse.bacc as bacc
    import concourse.mybir as mybir
    import concourse.tile as tile

    dt = mybir.dt
    F32 = dt.float32
    F32R = dt.float32r
    F16 = dt.float16
    AF = mybir.ActivationFunctionType
    ALU = mybir.AluOpType

    ncat = _pad128(ncat_real)
    nrem = _pad128(nrem_real)

    nc = bacc.Bacc("TRN2", target_bir_lowering=False, debug=False,
                   num_devices=NCORES)

    # ---------------- DRAM I/O ----------------
    catT_d = nc.dram_tensor("catT", [E, ncat], F16, kind="ExternalInput")
    remT_d = nc.dram_tensor("remT", [E, nrem], F16, kind="ExternalInput")
    wd = {}
    for l in ("1", "2", "c"):
        for p in ("q", "k", "v"):
            wd[p + l] = nc.dram_tensor(f"w{p}{l}", [E, DLOC], F16,
                                       kind="ExternalInput")
        wd["o" + l] = nc.dram_tensor(f"wo{l}", [DLOC, E], F16,
                                     kind="ExternalInput")
    wd["f1"] = nc.dram_tensor("wf1", [E, FLOC], F16, kind="ExternalInput")
    wd["f2"] = nc.dram_tensor("wf2", [FLOC, E], F16, kind="ExternalInput")
    # row-sums of W_qc (over E) and Wf1_shard (over E), for the LN identity
    wqcr_d = nc.dram_tensor("wqcr", [1, DLOC], F16, kind="ExternalInput")
    wf1r_d = nc.dram_tensor("wf1r", [1, FLOC], F16, kind="ExternalInput")
    wsb_d = nc.dram_tensor("wsb", [128, 4], F16, kind="ExternalInput")
    masks_d = nc.dram_tensor("masks", [128, 4], F16, kind="ExternalInput")
    consts_d = nc.dram_tensor("consts", [1, 4], F32, kind="ExternalInput")
    pstats_d = nc.dram_tensor("pstats", [3, nrem], F32, kind="ExternalOutput")
    dbg = {}
    if debug:
        for nm, L in (("dbg_x1", ncat), ("dbg_r", nrem), ("dbg_x2", nrem)):
            dbg[nm] = nc.dram_tensor(nm, [E, L], F16, kind="ExternalOutput")

    replica = [list(range(NCORES))]

    with tile.TileContext(nc, num_cores=NCORES) as tc:
        with (
            tc.tile_pool(name="acts", bufs=1) as acts,
            tc.tile_pool(name="psum", bufs=1, space="PSUM") as psum,
            tc.tile_pool(name="dram", bufs=1, space="DRAM") as dram,
        ):
            # ---- constants / packed stat tiles ----
            ones_col = acts.tile([128, 1], F16, name="ones_col",
                                 tag="ones_col")
            nc.vector.memset(ones_col[:], 1.0)
            ones_row = acts.tile([1, 128], F32R, name="ones_row",
                                 tag="ones_row")
            nc.vector.memset(ones_row[:].bitcast(F32), 1.0)
            masks = acts.tile([128, 4], F16, name="masks", tag="masks")
            nc.sync.dma_start(masks[:], masks_d.ap())
            consts = acts.tile([1, 4], F32, name="consts", tag="consts")
            nc.sync.dma_start(consts[:], consts_d.ap())
            wqcr = acts.tile([1, DLOC], F16, name="wqcr", tag="wqcr")
            nc.sync.dma_start(wqcr[:], wqcr_d.ap())
            wf1r = acts.tile([1, FLOC], F16, name="wf1r", tag="wf1r")
            nc.sync.dma_start(wf1r[:], wf1r_d.ap())

            def pp(name, L):
                return psum.tile([128, L], F32, name=name, tag="pp", bufs=8)

            def pstat(name, L):
                return psum.tile([1, L], F32, name=name, tag="pp", bufs=8)

            def wtile(name, cols):
                return acts.tile([128, cols], F16, name=name, tag="wt",
                                 bufs=10, padded_shape=[128, 1024])

            def wtile_r(name):
                """Resident [128,512] weight tile (full-sweep preload);
                1.5 sweeps of slots so the next sweep can prefetch."""
                return acts.tile([128, 512], F16, name=name, tag="wtr",
                                 bufs=48)

            def ar_bufs(tag, L, nchunks):
                rows = E // nchunks
                ins, outs = [], []
                for q in range(nchunks):
                    ins.append(dram.tile([rows, L], F16,
                                         name=f"arin{tag}_{q}",
                                         tag=f"arin{tag}_{q}"))
                    outs.append(dram.tile([rows, L], F16,
                                          name=f"arout{tag}_{q}",
                                          tag=f"arout{tag}_{q}",
                                          addr_space="Shared"))
                return ins, outs

            # ---------------- building blocks ----------------
            def load_xT(name, dram_t, L, tagbase):
                ts = []
                for k in range(ET):
                    xt = acts.tile([128, L], F16, name=f"{name}_{k}",
                                   tag=f"{tagbase}_{k}")
                    nc.sync.dma_start(xt[:],
                                      dram_t.ap()[128 * k:128 * (k + 1), :])
                    ts.append(xt)
                return ts

            def proj_fm(tagbase, w_dram, x_tiles, L, outtag):
                """q/k fm projection -> 4 tiles [128, L] (f16).
                m-outer / k-inner: 32 consecutive matmuls accumulate into
                the SAME psum bank (avoids HAM-throttle bank cycling)."""
                ps = [pp(f"ps_{tagbase}_{m}", L) for m in range(4)]
                wts = []
                for k in range(ET):
                    wt = wtile_r(f"w_{tagbase}_{k}")
                    nc.sync.dma_start(
                        wt[:], w_dram.ap()[128 * k:128 * (k + 1), :])
                    wts.append(wt)
                outs = []
                for m in range(4):
                    for k in range(ET):
                        nc.tensor.matmul(ps[m][:],
                                         wts[k][:, 128 * m:128 * (m + 1)],
                                         x_tiles[k][:],
                                         start=(k == 0), stop=(k == ET - 1))
                    o = acts.tile([128, L], F16, name=f"{tagbase}_{m}",
                                  tag=f"{outtag}_{m}")
                    nc.scalar.copy(o[:], ps[m][:])
                    outs.append(o)
                return outs

            def proj_tm(tagbase, w_dram, x_tiles, L):
                """v tm projection -> L//128 tiles [128, DLOC] (f16).
                j-outer / k-inner for same-bank accumulation streaks."""
                jt = L // 128
                ps = [pp(f"ps_{tagbase}_{j}", DLOC) for j in range(jt)]
                wts = []
                for k in range(ET):
                    wt = wtile_r(f"w_{tagbase}_{k}")
                    nc.sync.dma_start(
                        wt[:], w_dram.ap()[128 * k:128 * (k + 1), :])
                    wts.append(wt)
                outs = []
                for j in range(jt):
                    for k in range(ET):
                        nc.tensor.matmul(ps[j][:],
                                         x_tiles[k][:, 128 * j:128 * (j + 1)],
                                         wts[k][:],
                                         start=(k == 0), stop=(k == ET - 1))
                    o = acts.tile([128, DLOC], F16, name=f"{tagbase}_{j}",
                                  tag=f"v_{j}")
                    nc.scalar.copy(o[:], ps[j][:])
                    outs.append(o)
                return outs

            def attention(tag, qT, kT, vT, Lq, Lkv, kv_valid, mask_idx):
                jt = Lkv // 128
                oT = []
                for h in range(HL):
                    exps = []
                    for j in range(jt):
                        p = pp(f"ps_s_{tag}_{h}_{j}", Lq)
                        for c in range(2):
                            nc.tensor.matmul(
                                p[:],
                                kT[2 * h + c][:, 128 * j:128 * (j + 1)],
                                qT[2 * h + c][:],
                                start=(c == 0), stop=(c == 1))
                        e = acts.tile([128, Lq], F16,
                                      name=f"es_{tag}_{h}_{j}",
                                      tag=f"expS_{j}")
                        nc.scalar.activation(e[:], p[:], AF.Exp,
                                             scale=float(1.0 / np.sqrt(DH)))
                        exps.append(e)
                    dsum = pstat(f"ps_d_{tag}_{h}", Lq)
                    for j in range(jt):
                        if j == jt - 1 and kv_valid < Lkv:
                            col = masks[:, mask_idx:mask_idx + 1]
                        else:
                            col = ones_col[:]
                        nc.tensor.matmul(dsum[:], col, exps[j][:],
                                         start=(j == 0), stop=(j == jt - 1))
                    rec2 = acts.tile([1, Lq], F32R, name=f"rec2_{tag}_{h}",
                                     tag="rec2")
                    with nc.allow_low_precision(
                            reason="f32r output has f32 bits"):
                        nc.vector.reciprocal(rec2[:], dsum[:])
                    rrep_p = pp(f"ps_rr_{tag}_{h}", Lq)
                    nc.tensor.matmul(rrep_p[:], ones_row[:], rec2[:],
                                     start=True, stop=True)
                    rrep = acts.tile([128, Lq], F32, name=f"rr_{tag}_{h}",
                                     tag="rrep")
                    nc.scalar.copy(rrep[:], rrep_p[:])
                    for c in range(2):
                        po = pp(f"ps_o_{tag}_{h}_{c}", Lq)
                        for j in range(jt):
                            nc.tensor.matmul(
                                po[:],
                                vT[j][:, 256 * h + 128 * c:
                                      256 * h + 128 * (c + 1)],
                                exps[j][:],
                                start=(j == 0), stop=(j == jt - 1))
                        o = acts.tile([128, Lq], F16,
                                      name=f"oT_{tag}_{h}_{c}",
                                      tag=f"oT_{2 * h + c}")
                        nc.vector.tensor_tensor(o[:], po[:], rrep[:],
                                                ALU.mult)
                        oT.append(o)
                return oT

            def out_proj_ar(tag, oT, w_dram, ar_ins, ar_outs, Lq):
                """Out-projection by feature quarter; each AllReduce chunk
                issued as soon as its quarters are staged."""
                nchunks = len(ar_ins)
                qpc = 4 // nchunks  # quarters per chunk
                for quarter in range(4):
                    ch, qi = quarter // qpc, quarter % qpc
                    wo_t = []
                    for k in range(4):
                        wt = wtile(f"wo_{tag}_{quarter}_{k}", 1024)
                        nc.sync.dma_start(
                            wt[:],
                            w_dram.ap()[128 * k:128 * (k + 1),
                                        1024 * quarter:1024 * (quarter + 1)])
                        wo_t.append(wt)
                    ps = [pp(f"ps_op_{tag}_{quarter}_{mm}", Lq)
                          for mm in range(8)]
                    for mm in range(8):
                        for k in range(4):
                            nc.tensor.matmul(
                                ps[mm][:],
                                wo_t[k][:, 128 * mm:128 * (mm + 1)],
                                oT[k][:],
                                start=(k == 0), stop=(k == 3))
                    for mm in range(8):
                        st = acts.tile([128, Lq], F16,
                                       name=f"st_{tag}_{quarter}_{mm}",
                                       tag="stage", bufs=6)
                        nc.scalar.copy(st[:], ps[mm][:])
                        nc.sync.dma_start(
                            ar_ins[ch][1024 * qi + 128 * mm:
                                       1024 * qi + 128 * (mm + 1), :],
                            st[:])
                    if qi == qpc - 1:
                        nc.gpsimd.collective_compute(
                            "AllReduce", ALU.add, replica_groups=replica,
                            ins=[ar_ins[ch].opt()],
                            outs=[ar_outs[ch].opt()])

            def ln_stats_math(tag, s1p, s2p, L):
                """[1,L] LN stats from raw sum/sq-sum PSUMs.
                Returns (rstd F32R, nmr F32R, negmean16 F16)."""
                mean = acts.tile([1, L], F32, name=f"mean_{tag}",
                                 tag="lmean")
                var = acts.tile([1, L], F32, name=f"var_{tag}", tag="lvar")
                tmpa = acts.tile([1, L], F32, name=f"tmpa_{tag}", tag="ltmp")
                r0 = acts.tile([1, L], F32, name=f"r0_{tag}", tag="lr0")
                negmean16 = acts.tile([1, L], F16, name=f"nm16_{tag}",
                                      tag="nm16")
                nc.scalar.mul(mean[:], s1p[:], 1.0 / E)
                nc.scalar.mul(negmean16[:], s1p[:], -1.0 / E)
                nc.scalar.mul(var[:], s2p[:], 1.0 / E)
                nc.scalar.square(tmpa[:], mean[:])
                nc.vector.tensor_sub(var[:], var[:], tmpa[:])
                nc.vector.tensor_scalar_add(var[:], var[:], 1e-5)
                nc.scalar.sqrt(tmpa[:], var[:])
                nc.vector.reciprocal(r0[:], tmpa[:])
                nc.vector.tensor_tensor(tmpa[:], r0[:], r0[:], ALU.mult)
                nc.vector.tensor_tensor(tmpa[:], tmpa[:], var[:], ALU.mult)
                nc.vector.tensor_scalar(tmpa[:], tmpa[:], -0.5, 1.5, ALU.mult,
                                        ALU.add)
                rstd = acts.tile([1, L], F32R, name=f"rstd_{tag}", tag="rstd")
                nmr = acts.tile([1, L], F32R, name=f"nmr_{tag}", tag="nmr")
                nc.vector.tensor_tensor(rstd[:], r0[:], tmpa[:], ALU.mult)
                nc.vector.scalar_tensor_tensor(nmr[:], mean[:], -1.0, rstd[:],
                                               ALU.mult, ALU.mult)
                return rstd, nmr, negmean16

            def ln_broadcast(tag, rstd, nmr, L):
                """Broadcast [1,L] A=rstd, B=nmr to [128,L] via matmul."""
                Apsum = pp(f"ps_A_{tag}", L)
                nc.tensor.matmul(Apsum[:], ones_row[:], rstd[:], start=True,
                                 stop=True)
                Bpsum = pp(f"ps_B_{tag}", L)
                nc.tensor.matmul(Bpsum[:], ones_row[:], nmr[:], start=True,
                                 stop=True)
                Asb = acts.tile([128, L], F32, name=f"A_{tag}", tag="Asb")
                nc.scalar.copy(Asb[:], Apsum[:])
                Bsb = acts.tile([128, L], F32, name=f"B_{tag}", tag="Bsb")
                nc.scalar.copy(Bsb[:], Bpsum[:])
                return Asb, Bsb

            def residual_ln(tag, ar_outs, res_tiles, L, valid=None,
                            dump=None):
                """In-place: res_tiles[k] <- LN(sum_chunks(ar) + res)[k]."""
                nchunks = len(ar_outs)
                kc = ET // nchunks
                s1p = pstat(f"ps_s1_{tag}", L)
                s2p = pstat(f"ps_s2_{tag}", L)
                for blk in range(2):
                    ks = range(blk * 16, (blk + 1) * 16)
                    sqs = []
                    for k in ks:
                        q, mm = k // kc, k % kc
                        b = acts.tile([128, L], F16, name=f"arb_{tag}_{k}",
                                      tag="arb", bufs=8)
                        nc.sync.dma_start(
                            b[:], ar_outs[q][128 * mm:128 * (mm + 1), :])
                        nc.vector.tensor_tensor(res_tiles[k][:], b[:],
                                                res_tiles[k][:], ALU.add)
                        sq = acts.tile([128, L], F16, name=f"sq_{tag}_{k}",
                                       tag="sqt", bufs=16)
                        nc.scalar.square(sq[:], res_tiles[k][:])
                        sqs.append(sq)
                    for k in ks:
                        nc.tensor.matmul(s1p[:], ones_col[:],
                                         res_tiles[k][:],
                                         start=(k == 0), stop=(k == ET - 1))
                    for i, k in enumerate(ks):
                        nc.tensor.matmul(s2p[:], ones_col[:], sqs[i][:],
                                         start=(k == 0), stop=(k == ET - 1))
                rstd, nmr, _ = ln_stats_math(tag, s1p, s2p, L)
                Asb, Bsb = ln_broadcast(tag, rstd, nmr, L)
                for k in range(ET):
                    nc.vector.tensor_tensor(res_tiles[k][:], res_tiles[k][:],
                                            Asb[:], ALU.mult)
                    nc.vector.tensor_tensor(res_tiles[k][:], res_tiles[k][:],
                                            Bsb[:], ALU.add)
                    if valid is not None and valid < L:
                        nc.vector.memset(res_tiles[k][:, valid:L], 0.0)
                    if dump is not None:
                        nc.sync.dma_start(
                            dump.ap()[128 * k:128 * (k + 1), :],
                            res_tiles[k][:])
                return res_tiles

            def ln_u_sweep(tag, w_dram, wcol0, nouts, wr_tile, ar_outs,
                           res_tiles, L, with_stats, stats=None):
                """Accumulate U[m] = W[:, wcol0+128m cols] @ y where
                y[k] = res[k] + ar chunk, consuming AR chunks as they land.
                If with_stats, also accumulate s1/s2 and finish the LN stats;
                then add the rank-1 rowsum(W) (x) (-mean) term so that
                U * rstd = W @ LN(y).  Returns (U psums, (rstd,nmr,Asb,Bsb)).
                res_tiles are left holding un-normalized y."""
                ps = [pp(f"ps_{tag}_{m}", L) for m in range(nouts)]
                if with_stats:
                    s1p = pstat(f"ps_s1_{tag}", L)
                    s2p = pstat(f"ps_s2_{tag}", L)
                wts = []
                for k in range(ET):
                    wt = wtile_r(f"w_{tag}_{k}")
                    nc.sync.dma_start(
                        wt[:], w_dram.ap()[128 * k:128 * (k + 1),
                                           wcol0:wcol0 + 128 * nouts])
                    wts.append(wt)
                for q in range(NCH):
                    ks = range(q * KC, (q + 1) * KC)
                    if with_stats:
                        sqs = []
                        for k in ks:
                            b = acts.tile([128, L], F16,
                                          name=f"arb_{tag}_{k}",
                                          tag="arb", bufs=8)
                            nc.sync.dma_start(
                                b[:],
                                ar_outs[q][128 * (k % KC):
                                           128 * (k % KC + 1), :])
                            nc.vector.tensor_tensor(res_tiles[k][:], b[:],
                                                    res_tiles[k][:],
                                                    ALU.add)
                            sq = acts.tile([128, L], F16,
                                           name=f"sq_{tag}_{k}",
                                           tag="sqt", bufs=16)
                            nc.scalar.square(sq[:], res_tiles[k][:])
                            sqs.append(sq)
                        for k in ks:
                            nc.tensor.matmul(s1p[:], ones_col[:],
                                             res_tiles[k][:],
                                             start=(k == 0),
                                             stop=(k == ET - 1))
                        for i, k in enumerate(ks):
                            nc.tensor.matmul(s2p[:], ones_col[:],
                                             sqs[i][:],
                                             start=(k == 0),
                                             stop=(k == ET - 1))
                    for m in range(nouts):
                        for k in ks:
                            nc.tensor.matmul(ps[m][:],
                                             wts[k][:,
                                                    128 * m:128 * (m + 1)],
                                             res_tiles[k][:],
                                             start=(k == 0), stop=False)
                if with_stats:
                    rstd, nmr, negmean16 = ln_stats_math(tag, s1p, s2p, L)
                    Asb, Bsb = ln_broadcast(tag, rstd, nmr, L)
                    stats = (rstd, nmr, negmean16, Asb, Bsb)
                negmean16 = stats[2]
                for m in range(nouts):
                    nc.tensor.matmul(
                        ps[m][:],
                        wr_tile[0:1, wcol0 + 128 * m:wcol0 + 128 * (m + 1)],
                        negmean16[:],
                        start=False, stop=True)
                return ps, stats

            # ================= program =================
            # ---- MHA1 (cat self-attention) first; AR1 overlaps MHA2 ----
            c_t = load_xT("catT", catT_d, ncat, "b")
            q1 = proj_fm("q1", wd["q1"], c_t, ncat, "q")
            k1 = proj_fm("k1", wd["k1"], c_t, ncat, "k")
            v1 = proj_tm("v1", wd["v1"], c_t, ncat)
            a_t = load_xT("remT", remT_d, nrem, "a")
            o1 = attention("a1", q1, k1, v1, ncat, ncat, ncat_real, 0)
            # MHA2 projections issued before op1 so the tensor engine has
            # work while a1's softmax chain runs on vector/scalar
            q2 = proj_fm("q2", wd["q2"], a_t, nrem, "q")
            k2 = proj_fm("k2", wd["k2"], a_t, nrem, "k")
            v2 = proj_tm("v2", wd["v2"], a_t, nrem)
            arin1, arout1 = ar_bufs("1", ncat, 1)
            out_proj_ar("op1", o1, wd["o1"], arin1, arout1, ncat)
            o2 = attention("a2", q2, k2, v2, nrem, nrem, nrem_real, 1)
            arin2, arout2 = ar_bufs("2", nrem, NCH)
            out_proj_ar("op2", o2, wd["o2"], arin2, arout2, nrem)

            # ---- x1 = LN(AR1 + cat); kc/vc fill the AR2 window ----
            x1_t = residual_ln("x1", arout1, c_t, ncat, valid=ncat_real,
                               dump=dbg.get("dbg_x1"))
            kc = proj_fm("kc", wd["kc"], x1_t, ncat, "k")
            vc = proj_tm("vc", wd["vc"], x1_t, ncat)

            # ---- qc = Wqc @ LN(AR2 + rem) via the LN identity,
            #      consuming AR2 chunk by chunk ----
            psq, st2 = ln_u_sweep("qc", wd["qc"], 0, 4, wqcr, arout2,
                                  a_t, nrem, True)
            A2sb, B2sb = st2[3], st2[4]
            qc = []
            for m in range(4):
                o = acts.tile([128, nrem], F16, name=f"qc_{m}", tag=f"q_{m}")
                nc.vector.tensor_tensor(o[:], psq[m][:], A2sb[:], ALU.mult)
                qc.append(o)

            # ---- MHAc (q from r, kv from x1) ----
            oc = attention("ac", qc, kc, vc, nrem, ncat, ncat_real, 0)
            arinc, aroutc = ar_bufs("c", nrem, NCH)
            out_proj_ar("opc", oc, wd["oc"], arinc, aroutc, nrem)
            # r = LN(y2) in place (residual for x2); issued after opc so
            # these 64 vector ops overlap the ARc chunks instead of
            # blocking attention-ac's vector work
            for k in range(ET):
                nc.vector.tensor_tensor(a_t[k][:], a_t[k][:], A2sb[:],
                                        ALU.mult)
                nc.vector.tensor_tensor(a_t[k][:], a_t[k][:], B2sb[:],
                                        ALU.add)
                if dbg.get("dbg_r") is not None:
                    nc.sync.dma_start(
                        dbg["dbg_r"].ap()[128 * k:128 * (k + 1), :],
                        a_t[k][:])

            # ---- FFN f1 via the LN identity on y3 = ARc + r,
            #      consuming ARc chunk by chunk; two half-sweeps ----
            psfA, st3 = ln_u_sweep("f1A", wd["f1"], 0, 4, wf1r, aroutc,
                                   a_t, nrem, True)
            A3sb, B3sb = st3[3], st3[4]
            hT = []
            for m in range(4):
                hp = acts.tile([128, nrem], F16, name=f"hp_{m}",
                               tag="stage", bufs=6)
                nc.vector.tensor_tensor(hp[:], psfA[m][:], A3sb[:], ALU.mult)
                h = acts.tile([128, nrem], F16, name=f"hT_{m}", tag=f"v_{m}")
                nc.scalar.activation(h[:], hp[:], AF.Gelu)
                hT.append(h)
            psfB, _ = ln_u_sweep("f1B", wd["f1"], 512, 4, wf1r, aroutc,
                                 a_t, nrem, False, stats=st3)
            for m in range(4):
                hp = acts.tile([128, nrem], F16, name=f"hp_{m + 4}",
                               tag="stage", bufs=6)
                nc.vector.tensor_tensor(hp[:], psfB[m][:], A3sb[:], ALU.mult)
                h = acts.tile([128, nrem], F16, name=f"hT_{m + 4}",
                              tag=f"q_{m}")
                nc.scalar.activation(h[:], hp[:], AF.Gelu)
                hT.append(h)
            # x2 = LN(y3) in place (residual folded into FFN2 staging)
            for k in range(ET):
                nc.vector.tensor_tensor(a_t[k][:], a_t[k][:], A3sb[:],
                                        ALU.mult)
                nc.vector.tensor_tensor(a_t[k][:], a_t[k][:], B3sb[:],
                                        ALU.add)
                if dbg.get("dbg_x2") is not None:
                    nc.sync.dma_start(
                        dbg["dbg_x2"].ap()[128 * k:128 * (k + 1), :],
                        a_t[k][:])

            # ---- FFN f2 + fp16 ReduceScatter (2 chunks) ----
            HK = FLOC // 128  # 8
            rsin, rsout = [], []
            for ch in range(NCH):
                rsin.append(dram.tile([CROWS, nrem], F16, name=f"rsin_{ch}",
                                      tag=f"rsin_{ch}"))
                rsout.append(dram.tile([CROWS // NCORES, nrem], F16,
                                       name=f"rsout_{ch}",
                                       tag=f"rsout_{ch}"))
            for quarter in range(4):
                ch, qi = quarter // 2, quarter % 2
                ps = [pp(f"ps_f2_{quarter}_{mm}", nrem) for mm in range(8)]
                wf_t = []
                for k in range(HK):
                    wt = wtile(f"w_f2_{quarter}_{k}", 1024)
                    nc.sync.dma_start(
                        wt[:],
                        wd["f2"].ap()[128 * k:128 * (k + 1),
                                      1024 * quarter:1024 * (quarter + 1)])
                    wf_t.append(wt)
                for mm in range(8):
                    for k in range(HK):
                        nc.tensor.matmul(
                            ps[mm][:],
                            wf_t[k][:, 128 * mm:128 * (mm + 1)],
                            hT[k][:],
                            start=(k == 0), stop=(k == HK - 1))
                for mm in range(8):
                    m = 8 * quarter + mm
                    st = acts.tile([128, nrem], F16, name=f"st_f2_{m}",
                                   tag="stage", bufs=6)
                    nc.vector.scalar_tensor_tensor(
                        st[:], a_t[m][:], 1.0 / NCORES, ps[mm][:],
                        ALU.mult, ALU.add)
                    nc.sync.dma_start(
                        rsin[ch][1024 * qi + 128 * mm:
                                 1024 * qi + 128 * (mm + 1), :], st[:])
                if qi == 1:
                    nc.gpsimd.collective_compute(
                        "ReduceScatter", ALU.add, replica_groups=replica,
                        ins=[rsin[ch].opt()], outs=[rsout[ch].opt()])

            # ---- final LN stats from scattered y = x2 + ffn ----
            wsb_sb = acts.tile([128, 4], F16, name="wsb_sb", tag="ws_sb")
            nc.sync.dma_start(wsb_sb[:], wsb_d.ap())
            s1p = pstat("ps_rs1", nrem)
            s2p = pstat("ps_rs2", nrem)
            wsp = pstat("ps_rsw", nrem)
            for ch in range(NCH):
                for j in range(CROWS // NCORES // 128):  # 2
                    idx = 2 * ch + j
                    bt = acts.tile([128, nrem], F16, name=f"rsb_{idx}",
                                   tag="arb", bufs=8)
                    nc.gpsimd.dma_start(bt[:],
                                        rsout[ch][128 * j:128 * (j + 1), :])
                    nc.tensor.matmul(s1p[:], ones_col[:], bt[:],
                                     start=(idx == 0), stop=(idx == 3))
                    nc.tensor.matmul(wsp[:], wsb_sb[:, idx:idx + 1], bt[:],
                                     start=(idx == 0), stop=(idx == 3))
                    sq = acts.tile([128, nrem], F16, name=f"rssq_{idx}",
                                   tag="stage", bufs=6)
                    nc.scalar.square(sq[:], bt[:])
                    nc.tensor.matmul(s2p[:], ones_col[:], sq[:],
                                     start=(idx == 0), stop=(idx == 3))
            # per-core partial stats; the tiny cross-core sum + LN/logit
            # math happens on the host (saves the tail AllReduce)
            s1s = acts.tile([1, nrem], F32, name="s1s", tag="lmean")
            s2s = acts.tile([1, nrem], F32, name="s2s", tag="lvar")
            wss = acts.tile([1, nrem], F32, name="wss", tag="lr0")
            nc.vector.tensor_copy(s1s[:], s1p[:])
            nc.vector.tensor_copy(s2s[:], s2p[:])
            nc.vector.tensor_copy(wss[:], wsp[:])
            nc.sync.dma_start(pstats_d.ap()[0:1, :], s1s[:])
            nc.sync.dma_start(pstats_d.ap()[1:2, :], s2s[:])
            nc.sync.dma_start(pstats_d.ap()[2:3, :], wss[:])

    nc.compile()
    return nc


# ----------------------------------------------------------------------------
# host orchestration
# ----------------------------------------------------------------------------

def _prep_in_maps(vision_feature, text_embed, sel_idx, rem_idx, ncat, nrem,
                  Wqkv1, Wo1, Wqkv2, Wo2, Wqkvc, Woc, Wf1, Wf2, Ws):
    f16 = np.float16
    sel = vision_feature[sel_idx]
    rem = vision_feature[rem_idx]
    cat = np.concatenate([sel, text_embed], axis=0)
    catT = np.zeros((E, ncat), f16)
    catT[:, :cat.shape[0]] = cat.T
    remT = np.zeros((E, nrem), f16)
    remT[:, :rem.shape[0]] = rem.T

    ncat_real = cat.shape[0]
    nrem_real = rem.shape[0]
    masks = np.zeros((128, 4), f16)
    masks[:ncat_real - 128 * (ncat // 128 - 1), 0] = 1.0
    masks[:nrem_real - 128 * (nrem // 128 - 1), 1] = 1.0
    consts = np.zeros((1, 4), np.float32)
    consts[0, 0] = Ws.astype(np.float64).sum()

    in_maps = []
    for c in range(NCORES):
        hs = slice(DLOC * c, DLOC * (c + 1))
        fs = slice(FLOC * c, FLOC * (c + 1))
        # core c's Ws rows for RS chunk ch, sub-tile j: [2048ch+256c+128j, +128)
        wsb = np.stack(
            [Ws[0, CROWS * ch + 256 * c + 128 * j:
                CROWS * ch + 256 * c + 128 * (j + 1)]
             for ch in range(NCH) for j in range(2)], axis=1).astype(f16)
        m = {"catT": catT, "remT": remT, "masks": masks, "consts": consts,
             "wsb": np.ascontiguousarray(wsb)}
        for l, Wqkv, Wo in (("1", Wqkv1, Wo1), ("2", Wqkv2, Wo2),
                            ("c", Wqkvc, Woc)):
            Wq, Wk, Wv = Wqkv[:E], Wqkv[E:2 * E], Wqkv[2 * E:]
            m["wq" + l] = np.ascontiguousarray(Wq[hs].T.astype(f16))
            m["wk" + l] = np.ascontiguousarray(Wk[hs].T.astype(f16))
            m["wv" + l] = np.ascontiguousarray(Wv[hs].T.astype(f16))
            m["wo" + l] = np.ascontiguousarray(Wo[:, hs].T.astype(f16))
        m["wf1"] = np.ascontiguousarray(Wf1[fs].T.astype(f16))
        m["wf2"] = np.ascontiguousarray(Wf2[:, fs].T.astype(f16))
        # row-sums over E for the LN-identity rank-1 terms (fp32 accum)
        m["wqcr"] = Wqkvc[:E][hs].sum(axis=1).astype(f16)[None, :]
        m["wf1r"] = Wf1[fs].sum(axis=1).astype(f16)[None, :]
        in_maps.append(m)
    return in_maps


def run_device(in_maps, ncat_real, nrem_real, debug=False, trace=False):
    from concourse.bass_utils import run_bass_kernel_spmd

    key = (ncat_real, nrem_real, debug)
    if key not in _CACHE:
        _CACHE[key] = _build_device(ncat_real, nrem_real, debug=debug)
    nc = _CACHE[key]
    return run_bass_kernel_spmd(nc, in_maps, list(range(NCORES)), trace=trace)


def _kernel_impl(inputs, debug=False, trace=False):
    vision_feature = np.asarray(inputs["vision_feature"], np.float32)
    text_embed = np.asarray(inputs["text_embed"], np.float32)
    attention_mask = np.asarray(inputs["attention_mask"])

    biases_zero = all(
        not np.any(np.asarray(inputs[b]))
        for b in ("bqkv1", "bo1", "bqkv2", "bo2", "bqkvc", "boc",
                  "bf1", "bf2", "bs"))
    if (not bool(attention_mask.all())) or (not biases_zero):
        return _reference_np(**{k: np.asarray(v) for k, v in inputs.items()}), None

    t, sel_idx, rem_idx = _score_partition(vision_feature, text_embed,
                                           attention_mask)
    ncat_real = t + text_embed.shape[0]
    nrem_real = vision_feature.shape[0] - t
    kk = int(t * EXPAND)

    in_maps = _prep_in_maps(
        vision_feature, text_embed, sel_idx, rem_idx,
        _pad128(ncat_real), _pad128(nrem_real),
        np.asarray(inputs["Wqkv1"], np.float32),
        np.asarray(inputs["Wo1"], np.float32),
        np.asarray(inputs["Wqkv2"], np.float32),
        np.asarray(inputs["Wo2"], np.float32),
        np.asarray(inputs["Wqkvc"], np.float32),
        np.asarray(inputs["Woc"], np.float32),
        np.asarray(inputs["Wf1"], np.float32),
        np.asarray(inputs["Wf2"], np.float32),
        np.asarray(inputs["Ws"], np.float32))
    res = run_device(in_maps, ncat_real, nrem_real, debug=debug, trace=trace)
    ps = np.stack([np.asarray(res.results[c]["pstats"], np.float64)
                   for c in range(NCORES)]).sum(axis=0)
    s1, s2, wdot = ps[0], ps[1], ps[2]
    mean = s1 / E
    var = s2 / E - mean * mean
    rstd = 1.0 / np.sqrt(var + 1e-5)
    wssum = float(np.asarray(inputs["Ws"], np.float64).sum())
    logits = (wdot - mean * wssum) * rstd
    logits = logits[:nrem_real]
    es = (1.0 / (1.0 + np.exp(-logits.astype(np.float32))))
    ei = np.argsort(-es, kind="stable")[:kk]
    final = np.sort(np.concatenate([sel_idx, rem_idx[ei]]))
    return vision_feature[final], res


def kernel(**inputs):
    out, _ = _kernel_impl(inputs)
    return out


# revision 26
# speedup vs baseline: 1.2046x; 1.0313x over previous
"""Trainium2 Bass kernel for nn_CosSimRouter_learn_49778670960796.

Host: cosine-similarity scoring / sort / gather (tiny, shape-determining).
Device (8 NeuronCores, tensor-parallel over heads/hidden):
  3x MHA + FFN + logits; fp16 storage + matmuls (fp32 PSUM accum).
  Comm/compute overlap:
   - MHA1 (small) first; its AllReduce overlaps MHA2 (large).
   - AR2/ARc split into 2 feature-half chunks, issued as soon as their
     out-proj half is staged.
   - The consumers of AR2/ARc (cross-attn q-proj, FFN f1) use the
     affine-LN identity  W@LN(y) = (W@y + rowsum(W) (x) (-mean)) * rstd
     so their heavy weight sweeps consume un-normalized y chunk by chunk
     while the AllReduce is still in flight.
   - FFN2 output: fp16 ReduceScatter (2 chunks) + tiny stat AllReduce;
     logits from the affine-LN identity.
Host: top-k + final gather (exact rows of the input).
"""

import numpy as np

E = 4096
H = 16
HID = 8192
GAMMA = 0.2
TEMP = 0.05
EXPAND = 0.7
NCORES = 8
ET = E // 128  # 32 feature tiles
DH = E // H  # 256
HL = H // NCORES  # 2 heads per core
DLOC = HL * DH  # 512 local head dims
FLOC = HID // NCORES  # 1024 local ffn hidden
NCH = 2  # feature-half chunks per big collective
CROWS = E // NCH  # 2048
KC = ET // NCH  # 16 feature tiles per chunk

_CACHE = {}


# ----------------------------------------------------------------------------
# host-side reference math (numpy, fp32) for the scoring stage + fallback
# ----------------------------------------------------------------------------

def _score_partition(vision_feature, text_embed, attention_mask):
    vf = vision_feature.astype(np.float32)
    te = text_embed.astype(np.float32)
    vn = vf / np.maximum(np.linalg.norm(vf, axis=-1, keepdims=True), 1e-8)
    tn = te / np.maximum(np.linalg.norm(te, axis=-1, keepdims=True), 1e-8)
    cs = vn @ tn.T
    cs = np.where(attention_mask[None, :], cs, np.float32(0.0))
    m = cs.max(axis=-1) / np.float32(TEMP)
    e = np.exp(m - m.max())
    scores = e / e.sum()
    order = np.argsort(-scores, kind="stable")
    cum = np.cumsum(scores[order])
    t = int((cum <= GAMMA).sum())
    return t, order[:t], order[t:]


def _ln_np(x):
    m = x.mean(-1, keepdims=True)
    v = ((x - m) ** 2).mean(-1, keepdims=True)
    return (x - m) / np.sqrt(v + 1e-5)


def _gelu_np(x):
    import math

    erf = np.frompyfunc(math.erf, 1, 1)
    return (x * 0.5 * (1.0 + erf(x / math.sqrt(2.0)).astype(np.float64))
            ).astype(x.dtype)


def _mha_np(q_in, kv_in, Wqkv, bqkv, Wo, bo):
    dh = E // H
    Wq, Wk, Wv = np.split(Wqkv, 3, axis=0)
    bq, bk, bv = np.split(bqkv, 3)
    q = (q_in @ Wq.T + bq).reshape(-1, H, dh)
    k = (kv_in @ Wk.T + bk).reshape(-1, H, dh)
    v = (kv_in @ Wv.T + bv).reshape(-1, H, dh)
    att = np.einsum("qhd,khd->hqk", q, k) / np.float32(np.sqrt(dh))
    att = att - att.max(-1, keepdims=True)
    att = np.exp(att)
    att /= att.sum(-1, keepdims=True)
    o = np.einsum("hqk,khd->qhd", att.astype(np.float32), v).reshape(-1, E)
    return o @ Wo.T + bo


def _reference_np(vision_feature, text_embed, attention_mask,
                  Wqkv1, bqkv1, Wo1, bo1, Wqkv2, bqkv2, Wo2, bo2,
                  Wqkvc, bqkvc, Woc, boc, Wf1, bf1, Wf2, bf2, Ws, bs):
    t, sel_idx, rem_idx = _score_partition(vision_feature, text_embed,
                                           attention_mask)
    sel = vision_feature[sel_idx]
    rem = vision_feature[rem_idx]
    cat = np.concatenate([sel, text_embed], axis=0)
    x = _ln_np(_mha_np(cat, cat, Wqkv1, bqkv1, Wo1, bo1) + cat)
    r = _ln_np(_mha_np(rem, rem, Wqkv2, bqkv2, Wo2, bo2) + rem)
    x = _ln_np(_mha_np(r, x, Wqkvc, bqkvc, Woc, boc) + r)
    ffn = _gelu_np(x @ Wf1.T + bf1) @ Wf2.T + bf2
    x = _ln_np(x + ffn)
    logits = (x @ Ws.T + bs).squeeze(-1)
    es = 1.0 / (1.0 + np.exp(-logits))
    k = int(t * EXPAND)
    ei = np.argsort(-es, kind="stable")[:k]
    final = np.sort(np.concatenate([sel_idx, rem_idx[ei]]))
    return vision_feature[final]


# ----------------------------------------------------------------------------
# device program
# ----------------------------------------------------------------------------

def _pad128(n):
    return ((n + 127) // 128) * 128


def _build_device(ncat_real, nrem_real, debug=False):
    import concourse.bacc as bacc
    import concourse.mybir as mybir
    import concourse.tile as tile

    dt = mybir.dt
    F32 = dt.float32
    F32R = dt.float32r
    F16 = dt.float16
    AF = mybir.ActivationFunctionType
    ALU = mybir.AluOpType

    ncat = _pad128(ncat_real)
    nrem = _pad128(nrem_real)

    nc = bacc.Bacc("TRN2", target_bir_lowering=False, debug=False,
                   num_devices=NCORES)

    # ---------------- DRAM I/O ----------------
    catT_d = nc.dram_tensor("catT", [E, ncat], F16, kind="ExternalInput")
    remT_d = nc.dram_tensor("remT", [E, nrem], F16, kind="ExternalInput")
    wd = {}
    for l in ("1", "2", "c"):
        for p in ("q", "k", "v"):
            wd[p + l] = nc.dram_tensor(f"w{p}{l}", [E, DLOC], F16,
                                       kind="ExternalInput")
        wd["o" + l] = nc.dram_tensor(f"wo{l}", [DLOC, E], F16,
                                     kind="ExternalInput")
    wd["f1"] = nc.dram_tensor("wf1", [E, FLOC], F16, kind="ExternalInput")
    wd["f2"] = nc.dram_tensor("wf2", [FLOC, E], F16, kind="ExternalInput")
    # row-sums of W_qc (over E) and Wf1_shard (over E), for the LN identity
    wqcr_d = nc.dram_tensor("wqcr", [1, DLOC], F16, kind="ExternalInput")
    wf1r_d = nc.dram_tensor("wf1r", [1, FLOC], F16, kind="ExternalInput")
    wsb_d = nc.dram_tensor("wsb", [128, 4], F16, kind="ExternalInput")
    masks_d = nc.dram_tensor("masks", [128, 4], F16, kind="ExternalInput")
    consts_d = nc.dram_tensor("consts", [1, 4], F32, kind="ExternalInput")
    pstats_d = nc.dram_tensor("pstats", [3, nrem], F32, kind="ExternalOutput")
    dbg = {}
    if debug:
        for nm, L in (("dbg_x1", ncat), ("dbg_r", nrem), ("dbg_x2", nrem)):
            dbg[nm] = nc.dram_tensor(nm, [E, L], F16, kind="ExternalOutput")

    replica = [list(range(NCORES))]

    with tile.TileContext(nc, num_cores=NCORES) as tc:
        with (
            tc.tile_pool(name="acts", bufs=1) as acts,
            tc.tile_pool(name="psum", bufs=1, space="PSUM") as psum,
            tc.tile_pool(name="dram", bufs=1, space="DRAM") as dram,
        ):
            # ---- constants / packed stat tiles ----
            ones_col = acts.tile([128, 1], F16, name="ones_col",
                                 tag="ones_col")
            nc.vector.memset(ones_col[:], 1.0)
            ones_row = acts.tile([1, 128], F32R, name="ones_row",
                                 tag="ones_row")
            nc.vector.memset(ones_row[:].bitcast(F32), 1.0)
            masks = acts.tile([128, 4], F16, name="masks", tag="masks")
            nc.sync.dma_start(masks[:], masks_d.ap())
            consts = acts.tile([1, 4], F32, name="consts", tag="consts")
            nc.sync.dma_start(consts[:], consts_d.ap())
            wqcr = acts.tile([1, DLOC], F16, name="wqcr", tag="wqcr")
            nc.sync.dma_start(wqcr[:], wqcr_d.ap())
            wf1r = acts.tile([1, FLOC], F16, name="wf1r", tag="wf1r")
            nc.sync.dma_start(wf1r[:], wf1r_d.ap())

            def pp(name, L):
                return psum.tile([128, L], F32, name=name, tag="pp", bufs=8)

            def pstat(name, L):
                return psum.tile([1, L], F32, name=name, tag="pp", bufs=8)

            def wtile(name, cols):
                return acts.tile([128, cols], F16, name=name, tag="wt",
                                 bufs=10, padded_shape=[128, 1024])

            def wtile_r(name):
                """Resident [128,512] weight tile (full-sweep preload);
                1.5 sweeps of slots so the next sweep can prefetch."""
                return acts.tile([128, 512], F16, name=name, tag="wtr",
                                 bufs=48)

            def ar_bufs(tag, L, nchunks):
                rows = E // nchunks
                ins, outs = [], []
                for q in range(nchunks):
                    ins.append(dram.tile([rows, L], F16,
                                         name=f"arin{tag}_{q}",
                                         tag=f"arin{tag}_{q}"))
                    outs.append(dram.tile([rows, L], F16,
                                          name=f"arout{tag}_{q}",
                                          tag=f"arout{tag}_{q}",
                                          addr_space="Shared"))
                return ins, outs

            # ---------------- building blocks ----------------
            def load_xT(name, dram_t, L, tagbase):
                ts = []
                for k in range(ET):
                    xt = acts.tile([128, L], F16, name=f"{name}_{k}",
                                   tag=f"{tagbase}_{k}")
                    nc.sync.dma_start(xt[:],
                                      dram_t.ap()[128 * k:128 * (k + 1), :])
                    ts.append(xt)
                return ts

            def proj_fm(tagbase, w_dram, x_tiles, L, outtag):
                """q/k fm projection -> 4 tiles [128, L] (f16).
                m-outer / k-inner: 32 consecutive matmuls accumulate into
                the SAME psum bank (avoids HAM-throttle bank cycling)."""
                ps = [pp(f"ps_{tagbase}_{m}", L) for m in range(4)]
                wts = []
                for k in range(ET):
                    wt = wtile_r(f"w_{tagbase}_{k}")
                    nc.sync.dma_start(
                        wt[:], w_dram.ap()[128 * k:128 * (k + 1), :])
                    wts.append(wt)
                outs = []
                for m in range(4):
                    for k in range(ET):
                        nc.tensor.matmul(ps[m][:],
                                         wts[k][:, 128 * m:128 * (m + 1)],
                                         x_tiles[k][:],
                                         start=(k == 0), stop=(k == ET - 1))
                    o = acts.tile([128, L], F16, name=f"{tagbase}_{m}",
                                  tag=f"{outtag}_{m}")
                    nc.scalar.copy(o[:], ps[m][:])
                    outs.append(o)
                return outs

            def proj_tm(tagbase, w_dram, x_tiles, L):
                """v tm projection -> L//128 tiles [128, DLOC] (f16).
                j-outer / k-inner for same-bank accumulation streaks."""
                jt = L // 128
                ps = [pp(f"ps_{tagbase}_{j}", DLOC) for j in range(jt)]
                wts = []
                for k in range(ET):
                    wt = wtile_r(f"w_{tagbase}_{k}")
                    nc.sync.dma_start(
                        wt[:], w_dram.ap()[128 * k:128 * (k + 1), :])
                    wts.append(wt)
                outs = []
                for j in range(jt):
                    for k in range(ET):
                        nc.tensor.matmul(ps[j][:],
                                         x_tiles[k][:, 128 * j:128 * (j + 1)],
                                         wts[k][:],
                                         start=(k == 0), stop=(k == ET - 1))
                    o = acts.tile([128, DLOC], F16, name=f"{tagbase}_{j}",
                                  tag=f"v_{j}")
                    nc.scalar.copy(o[:], ps[j][:])
                    outs.append(o)
                return outs

            def attention(tag, qT, kT, vT, Lq, Lkv, kv_valid, mask_idx):
                jt = Lkv // 128
                oT = []
                for h in range(HL):
                    exps = []
                    for j in range(jt):
                        p = pp(f"ps_s_{tag}_{h}_{j}", Lq)
                        for c in range(2):
                            nc.tensor.matmul(
                                p[:],
                                kT[2 * h + c][:, 128 * j:128 * (j + 1)],
                                qT[2 * h + c][:],
                                start=(c == 0), stop=(c == 1))
                        e = acts.tile([128, Lq], F16,
                                      name=f"es_{tag}_{h}_{j}",
                                      tag=f"expS_{j}")
                        nc.scalar.activation(e[:], p[:], AF.Exp,
                                             scale=float(1.0 / np.sqrt(DH)))
                        exps.append(e)
                    dsum = pstat(f"ps_d_{tag}_{h}", Lq)
                    for j in range(jt):
                        if j == jt - 1 and kv_valid < Lkv:
                            col = masks[:, mask_idx:mask_idx + 1]
                        else:
                            col = ones_col[:]
                        nc.tensor.matmul(dsum[:], col, exps[j][:],
                                         start=(j == 0), stop=(j == jt - 1))
                    rec2 = acts.tile([1, Lq], F32R, name=f"rec2_{tag}_{h}",
                                     tag="rec2")
                    with nc.allow_low_precision(
                            reason="f32r output has f32 bits"):
                        nc.vector.reciprocal(rec2[:], dsum[:])
                    rrep_p = pp(f"ps_rr_{tag}_{h}", Lq)
                    nc.tensor.matmul(rrep_p[:], ones_row[:], rec2[:],
                                     start=True, stop=True)
                    rrep = acts.tile([128, Lq], F32, name=f"rr_{tag}_{h}",
                                     tag="rrep")
                    nc.scalar.copy(rrep[:], rrep_p[:])
                    for c in range(2):
                        po = pp(f"ps_o_{tag}_{h}_{c}", Lq)
                        for j in range(jt):
                            nc.tensor.matmul(
                                po[:],
                                vT[j][:, 256 * h + 128 * c:
                                      256 * h + 128 * (c + 1)],
                                exps[j][:],
                                start=(j == 0), stop=(j == jt - 1))
                        o = acts.tile([128, Lq], F16,
                                      name=f"oT_{tag}_{h}_{c}",
                                      tag=f"oT_{2 * h + c}")
                        nc.vector.tensor_tensor(o[:], po[:], rrep[:],
                                                ALU.mult)
                        oT.append(o)
                return oT

            def out_proj_ar(tag, oT, w_dram, ar_ins, ar_outs, Lq):
                """Out-projection by feature quarter; each AllReduce chunk
                issued as soon as its quarters are staged."""
                nchunks = len(ar_ins)
                qpc = 4 // nchunks  # quarters per chunk
                for quarter in range(4):
                    ch, qi = quarter // qpc, quarter % qpc
                    wo_t = []
                    for k in range(4):
                        wt = wtile(f"wo_{tag}_{quarter}_{k}", 1024)
                        nc.sync.dma_start(
                            wt[:],
                            w_dram.ap()[128 * k:128 * (k + 1),
                                        1024 * quarter:1024 * (quarter + 1)])
                        wo_t.append(wt)
                    ps = [pp(f"ps_op_{tag}_{quarter}_{mm}", Lq)
                          for mm in range(8)]
                    for mm in range(8):
                        for k in range(4):
                            nc.tensor.matmul(
                                ps[mm][:],
                                wo_t[k][:, 128 * mm:128 * (mm + 1)],
                                oT[k][:],
                                start=(k == 0), stop=(k == 3))
                    for mm in range(8):
                        st = acts.tile([128, Lq], F16,
                                       name=f"st_{tag}_{quarter}_{mm}",
                                       tag="stage", bufs=6)
                        nc.scalar.copy(st[:], ps[mm][:])
                        nc.sync.dma_start(
                            ar_ins[ch][1024 * qi + 128 * mm:
                                       1024 * qi + 128 * (mm + 1), :],
                            st[:])
                    if qi == qpc - 1:
                        nc.gpsimd.collective_compute(
                            "AllReduce", ALU.add, replica_groups=replica,
                            ins=[ar_ins[ch].opt()],
                            outs=[ar_outs[ch].opt()])

            def ln_stats_math(tag, s1p, s2p, L):
                """[1,L] LN stats from raw sum/sq-sum PSUMs.
                Returns (rstd F32R, nmr F32R, negmean16 F16)."""
                mean = acts.tile([1, L], F32, name=f"mean_{tag}",
                                 tag="lmean")
                var = acts.tile([1, L], F32, name=f"var_{tag}", tag="lvar")
                tmpa = acts.tile([1, L], F32, name=f"tmpa_{tag}", tag="ltmp")
                r0 = acts.tile([1, L], F32, name=f"r0_{tag}", tag="lr0")
                negmean16 = acts.tile([1, L], F16, name=f"nm16_{tag}",
                                      tag="nm16")
                nc.scalar.mul(mean[:], s1p[:], 1.0 / E)
                nc.scalar.mul(negmean16[:], s1p[:], -1.0 / E)
                nc.scalar.mul(var[:], s2p[:], 1.0 / E)
                nc.scalar.square(tmpa[:], mean[:])
                nc.vector.tensor_sub(var[:], var[:], tmpa[:])
                nc.vector.tensor_scalar_add(var[:], var[:], 1e-5)
                nc.scalar.sqrt(tmpa[:], var[:])
                nc.vector.reciprocal(r0[:], tmpa[:])
                nc.vector.tensor_tensor(tmpa[:], r0[:], r0[:], ALU.mult)
                nc.vector.tensor_tensor(tmpa[:], tmpa[:], var[:], ALU.mult)
                nc.vector.tensor_scalar(tmpa[:], tmpa[:], -0.5, 1.5, ALU.mult,
                                        ALU.add)
                rstd = acts.tile([1, L], F32R, name=f"rstd_{tag}", tag="rstd")
                nmr = acts.tile([1, L], F32R, name=f"nmr_{tag}", tag="nmr")
                nc.vector.tensor_tensor(rstd[:], r0[:], tmpa[:], ALU.mult)
                nc.vector.scalar_tensor_tensor(nmr[:], mean[:], -1.0, rstd[:],
                                               ALU.mult, ALU.mult)
                return rstd, nmr, negmean16

            def ln_broadcast(tag, rstd, nmr, L):
                """Broadcast [1,L] A=rstd, B=nmr to [128,L] via matmul."""
                Apsum = pp(f"ps_A_{tag}", L)
                nc.tensor.matmul(Apsum[:], ones_row[:], rstd[:], start=True,
                                 stop=True)
                Bpsum = pp(f"ps_B_{tag}", L)
                nc.tensor.matmul(Bpsum[:], ones_row[:], nmr[:], start=True,
                                 stop=True)
                Asb = acts.tile([128, L], F32, name=f"A_{tag}", tag="Asb")
                nc.scalar.copy(Asb[:], Apsum[:])
                Bsb = acts.tile([128, L], F32, name=f"B_{tag}", tag="Bsb")
                nc.scalar.copy(Bsb[:], Bpsum[:])
                return Asb, Bsb

            def residual_ln(tag, ar_outs, res_tiles, L, valid=None,
                            dump=None):
                """In-place: res_tiles[k] <- LN(sum_chunks(ar) + res)[k]."""
                nchunks = len(ar_outs)
                kc = ET // nchunks
                s1p = pstat(f"ps_s1_{tag}", L)
                s2p = pstat(f"ps_s2_{tag}", L)
                for blk in range(2):
                    ks = range(blk * 16, (blk + 1) * 16)
                    sqs = []
                    for k in ks:
                        q, mm = k // kc, k % kc
                        b = acts.tile([128, L], F16, name=f"arb_{tag}_{k}",
                                      tag="arb", bufs=8)
                        nc.sync.dma_start(
                            b[:], ar_outs[q][128 * mm:128 * (mm + 1), :])
                        nc.vector.tensor_tensor(res_tiles[k][:], b[:],
                                                res_tiles[k][:], ALU.add)
                        sq = acts.tile([128, L], F16, name=f"sq_{tag}_{k}",
                                       tag="sqt", bufs=16)
                        nc.scalar.square(sq[:], res_tiles[k][:])
                        sqs.append(sq)
                    for k in ks:
                        nc.tensor.matmul(s1p[:], ones_col[:],
                                         res_tiles[k][:],
                                         start=(k == 0), stop=(k == ET - 1))
                    for i, k in enumerate(ks):
                        nc.tensor.matmul(s2p[:], ones_col[:], sqs[i][:],
                                         start=(k == 0), stop=(k == ET - 1))
                rstd, nmr, _ = ln_stats_math(tag, s1p, s2p, L)
                Asb, Bsb = ln_broadcast(tag, rstd, nmr, L)
                for k in range(ET):
                    nc.vector.tensor_tensor(res_tiles[k][:], res_tiles[k][:],
                                            Asb[:], ALU.mult)
                    nc.vector.tensor_tensor(res_tiles[k][:], res_tiles[k][:],
                                            Bsb[:], ALU.add)
                    if valid is not None and valid < L:
                        nc.vector.memset(res_tiles[k][:, valid:L], 0.0)
                    if dump is not None:
                        nc.sync.dma_start(
                            dump.ap()[128 * k:128 * (k + 1), :],
                            res_tiles[k][:])
                return res_tiles

            def ln_u_sweep(tag, w_dram, wcol0, nouts, wr_tile, ar_outs,
                           res_tiles, L, with_stats, stats=None):
                """Accumulate U[m] = W[:, wcol0+128m cols] @ y where
                y[k] = res[k] + ar chunk, consuming AR chunks as they land.
                If with_stats, also accumulate s1/s2 and finish the LN stats;
                then add the rank-1 rowsum(W) (x) (-mean) term so that
                U * rstd = W @ LN(y).  Returns (U psums, (rstd,nmr,Asb,Bsb)).
                res_tiles are left holding un-normalized y."""
                ps = [pp(f"ps_{tag}_{m}", L) for m in range(nouts)]
                if with_stats:
                    s1p = pstat(f"ps_s1_{tag}", L)
                    s2p = pstat(f"ps_s2_{tag}", L)
                wts = []
                for k in range(ET):
                    wt = wtile_r(f"w_{tag}_{k}")
                    nc.sync.dma_start(
                        wt[:], w_dram.ap()[128 * k:128 * (k + 1),
                                           wcol0:wcol0 + 128 * nouts])
                    wts.append(wt)
                for q in range(NCH):
                    ks = range(q * KC, (q + 1) * KC)
                    if with_stats:
                        sqs = []
                        for k in ks:
                            b = acts.tile([128, L], F16,
                                          name=f"arb_{tag}_{k}",
                                          tag="arb", bufs=8)
                            nc.sync.dma_start(
                                b[:],
                                ar_outs[q][128 * (k % KC):
                                           128 * (k % KC + 1), :])
                            nc.vector.tensor_tensor(res_tiles[k][:], b[:],
                                                    res_tiles[k][:],
                                                    ALU.add)
                            sq = acts.tile([128, L], F16,
                                           name=f"sq_{tag}_{k}",
                                           tag="sqt", bufs=16)
                            nc.scalar.square(sq[:], res_tiles[k][:])
                            sqs.append(sq)
                        for k in ks:
                            nc.tensor.matmul(s1p[:], ones_col[:],
                                             res_tiles[k][:],
                                             start=(k == 0),
                                             stop=(k == ET - 1))
                        for i, k in enumerate(ks):
                            nc.tensor.matmul(s2p[:], ones_col[:],
                                             sqs[i][:],
                                             start=(k == 0),
                                             stop=(k == ET - 1))
                    for m in range(nouts):
                        for k in ks:
                            nc.tensor.matmul(ps[m][:],
                                             wts[k][:,
                                                    128 * m:128 * (m + 1)],
                                             res_tiles[k][:],
                                             start=(k == 0), stop=False)
                if with_stats:
                    rstd, nmr, negmean16 = ln_stats_math(tag, s1p, s2p, L)
                    Asb, Bsb = ln_broadcast(tag, rstd, nmr, L)
                    stats = (rstd, nmr, negmean16, Asb, Bsb)
                negmean16 = stats[2]
                for m in range(nouts):
                    nc.tensor.matmul(
                        ps[m][:],
                        wr_tile[0:1, wcol0 + 128 * m:wcol0 + 128 * (m + 1)],
                        negmean16[:],
                        start=False, stop=True)
                return ps, stats

            # ================= program =================
            # ---- MHA1 (cat self-attention) first; AR1 overlaps MHA2 ----
            c_t = load_xT("catT", catT_d, ncat, "b")
            q1 = proj_fm("q1", wd["q1"], c_t, ncat, "q")
            k1 = proj_fm("k1", wd["k1"], c_t, ncat, "k")
            v1 = proj_tm("v1", wd["v1"], c_t, ncat)
            a_t = load_xT("remT", remT_d, nrem, "a")
            o1 = attention("a1", q1, k1, v1, ncat, ncat, ncat_real, 0)
            # v2-proj fills the tensor queue while a1's softmax chain runs
            # on vector/scalar; op1 right after so AR1 starts early and
            # overlaps the q2/k2 sweeps
            v2 = proj_tm("v2", wd["v2"], a_t, nrem)
            arin1, arout1 = ar_bufs("1", ncat, 1)
            out_proj_ar("op1", o1, wd["o1"], arin1, arout1, ncat)
            q2 = proj_fm("q2", wd["q2"], a_t, nrem, "q")
            k2 = proj_fm("k2", wd["k2"], a_t, nrem, "k")
            o2 = attention("a2", q2, k2, v2, nrem, nrem, nrem_real, 1)
            arin2, arout2 = ar_bufs("2", nrem, NCH)
            out_proj_ar("op2", o2, wd["o2"], arin2, arout2, nrem)

            # ---- x1 = LN(AR1 + cat); with kc/vc it fills the AR2 window ----
            x1_t = residual_ln("x1", arout1, c_t, ncat, valid=ncat_real,
                               dump=dbg.get("dbg_x1"))
            kc = proj_fm("kc", wd["kc"], x1_t, ncat, "k")
            vc = proj_tm("vc", wd["vc"], x1_t, ncat)

            # ---- qc = Wqc @ LN(AR2 + rem) via the LN identity,
            #      consuming AR2 chunk by chunk ----
            psq, st2 = ln_u_sweep("qc", wd["qc"], 0, 4, wqcr, arout2,
                                  a_t, nrem, True)
            A2sb, B2sb = st2[3], st2[4]
            qc = []
            for m in range(4):
                o = acts.tile([128, nrem], F16, name=f"qc_{m}", tag=f"q_{m}")
                nc.vector.tensor_tensor(o[:], psq[m][:], A2sb[:], ALU.mult)
                qc.append(o)

            # ---- MHAc (q from r, kv from x1) ----
            oc = attention("ac", qc, kc, vc, nrem, ncat, ncat_real, 0)
            arinc, aroutc = ar_bufs("c", nrem, NCH)
            out_proj_ar("opc", oc, wd["oc"], arinc, aroutc, nrem)
            # r = LN(y2) in place (residual for x2); issued after opc so
            # these 64 vector ops overlap the ARc chunks instead of
            # blocking attention-ac's vector work
            for k in range(ET):
                nc.vector.tensor_tensor(a_t[k][:], a_t[k][:], A2sb[:],
                                        ALU.mult)
                nc.vector.tensor_tensor(a_t[k][:], a_t[k][:], B2sb[:],
                                        ALU.add)
                if dbg.get("dbg_r") is not None:
                    nc.sync.dma_start(
                        dbg["dbg_r"].ap()[128 * k:128 * (k + 1), :],
                        a_t[k][:])

            # ---- FFN f1 via the LN identity on y3 = ARc + r,
            #      consuming ARc chunk by chunk; two half-sweeps ----
            psfA, st3 = ln_u_sweep("f1A", wd["f1"], 0, 4, wf1r, aroutc,
                                   a_t, nrem, True)
            A3sb, B3sb = st3[3], st3[4]
            hT = []
            for m in range(4):
                hp = acts.tile([128, nrem], F16, name=f"hp_{m}",
                               tag="stage", bufs=6)
                nc.vector.tensor_tensor(hp[:], psfA[m][:], A3sb[:], ALU.mult)
                h = acts.tile([128, nrem], F16, name=f"hT_{m}", tag=f"v_{m}")
                nc.scalar.activation(h[:], hp[:], AF.Gelu)
                hT.append(h)
            psfB, _ = ln_u_sweep("f1B", wd["f1"], 512, 4, wf1r, aroutc,
                                 a_t, nrem, False, stats=st3)
            for m in range(4):
                hp = acts.tile([128, nrem], F16, name=f"hp_{m + 4}",
                               tag="stage", bufs=6)
                nc.vector.tensor_tensor(hp[:], psfB[m][:], A3sb[:], ALU.mult)
                h = acts.tile([128, nrem], F16, name=f"hT_{m + 4}",
                              tag=f"q_{m}")
                nc.scalar.activation(h[:], hp[:], AF.Gelu)
                hT.append(h)
            # x2 = LN(y3) in place (residual folded into FFN2 staging)
            for k in range(ET):
                nc.vector.tensor_tensor(a_t[k][:], a_t[k][:], A3sb[:],
                                        ALU.mult)
                nc.vector.tensor_tensor(a_t[k][:], a_t[k][:], B3sb[:],
                                        ALU.add)
                if dbg.get("dbg_x2") is not None:
                    nc.sync.dma_start(
                        dbg["dbg_x2"].ap()[128 * k:128 * (k + 1), :],
                        a_t[k][:])

            # ---- FFN f2 + fp16 ReduceScatter (2 chunks) ----
            HK = FLOC // 128  # 8
            rsin, rsout = [], []
            for ch in range(NCH):
                rsin.append(dram.tile([CROWS, nrem], F16, name=f"rsin_{ch}",
                                      tag=f"rsin_{ch}"))
                rsout.append(dram.tile([CROWS // NCORES, nrem], F16,
                                       name=f"rsout_{ch}",
                                       tag=f"rsout_{ch}"))
            for quarter in range(4):
                ch, qi = quarter // 2, quarter % 2
                ps = [pp(f"ps_f2_{quarter}_{mm}", nrem) for mm in range(8)]
                wf_t = []
                for k in range(HK):
                    wt = wtile(f"w_f2_{quarter}_{k}", 1024)
                    nc.sync.dma_start(
                        wt[:],
                        wd["f2"].ap()[128 * k:128 * (k + 1),
                                      1024 * quarter:1024 * (quarter + 1)])
                    wf_t.append(wt)
                for mm in range(8):
                    for k in range(HK):
                        nc.tensor.matmul(
                            ps[mm][:],
                            wf_t[k][:, 128 * mm:128 * (mm + 1)],
                            hT[k][:],
                            start=(k == 0), stop=(k == HK - 1))
                for mm in range(8):
                    m = 8 * quarter + mm
                    st = acts.tile([128, nrem], F16, name=f"st_f2_{m}",
                                   tag="stage", bufs=6)
                    nc.vector.scalar_tensor_tensor(
                        st[:], a_t[m][:], 1.0 / NCORES, ps[mm][:],
                        ALU.mult, ALU.add)
                    nc.sync.dma_start(
                        rsin[ch][1024 * qi + 128 * mm:
                                 1024 * qi + 128 * (mm + 1), :], st[:])
                if qi == 1:
                    nc.gpsimd.collective_compute(
                        "ReduceScatter", ALU.add, replica_groups=replica,
                        ins=[rsin[ch].opt()], outs=[rsout[ch].opt()])

            # ---- final LN stats from scattered y = x2 + ffn ----
            wsb_sb = acts.tile([128, 4], F16, name="wsb_sb", tag="ws_sb")
            nc.sync.dma_start(wsb_sb[:], wsb_d.ap())
            s1p = pstat("ps_rs1", nrem)
            s2p = pstat("ps_rs2", nrem)
            wsp = pstat("ps_rsw", nrem)
            for ch in range(NCH):
                for j in range(CROWS // NCORES // 128):  # 2
                    idx = 2 * ch + j
                    bt = acts.tile([128, nrem], F16, name=f"rsb_{idx}",
                                   tag="arb", bufs=8)
                    nc.gpsimd.dma_start(bt[:],
                                        rsout[ch][128 * j:128 * (j + 1), :])
                    nc.tensor.matmul(s1p[:], ones_col[:], bt[:],
                                     start=(idx == 0), stop=(idx == 3))
                    nc.tensor.matmul(wsp[:], wsb_sb[:, idx:idx + 1], bt[:],
                                     start=(idx == 0), stop=(idx == 3))
                    sq = acts.tile([128, nrem], F16, name=f"rssq_{idx}",
                                   tag="stage", bufs=6)
                    nc.scalar.square(sq[:], bt[:])
                    nc.tensor.matmul(s2p[:], ones_col[:], sq[:],
                                     start=(idx == 0), stop=(idx == 3))
            # per-core partial stats; the tiny cross-core sum + LN/logit
            # math happens on the host (saves the tail AllReduce)
            s1s = acts.tile([1, nrem], F32, name="s1s", tag="lmean")
            s2s = acts.tile([1, nrem], F32, name="s2s", tag="lvar")
            wss = acts.tile([1, nrem], F32, name="wss", tag="lr0")
            nc.vector.tensor_copy(s1s[:], s1p[:])
            nc.vector.tensor_copy(s2s[:], s2p[:])
            nc.vector.tensor_copy(wss[:], wsp[:])
            nc.sync.dma_start(pstats_d.ap()[0:1, :], s1s[:])
            nc.sync.dma_start(pstats_d.ap()[1:2, :], s2s[:])
            nc.sync.dma_start(pstats_d.ap()[2:3, :], wss[:])

    nc.compile()
    return nc


# ----------------------------------------------------------------------------
# host orchestration
# ----------------------------------------------------------------------------

def _prep_in_maps(vision_feature, text_embed, sel_idx, rem_idx, ncat, nrem,
                  Wqkv1, Wo1, Wqkv2, Wo2, Wqkvc, Woc, Wf1, Wf2, Ws):
    f16 = np.float16
    sel = vision_feature[sel_idx]
    rem = vision_feature[rem_idx]
    cat = np.concatenate([sel, text_embed], axis=0)
    catT = np.zeros((E, ncat), f16)
    catT[:, :cat.shape[0]] = cat.T
    remT = np.zeros((E, nrem), f16)
    remT[:, :rem.shape[0]] = rem.T

    ncat_real = cat.shape[0]
    nrem_real = rem.shape[0]
    masks = np.zeros((128, 4), f16)
    masks[:ncat_real - 128 * (ncat // 128 - 1), 0] = 1.0
    masks[:nrem_real - 128 * (nrem // 128 - 1), 1] = 1.0
    consts = np.zeros((1, 4), np.float32)
    consts[0, 0] = Ws.astype(np.float64).sum()

    in_maps = []
    for c in range(NCORES):
        hs = slice(DLOC * c, DLOC * (c + 1))
        fs = slice(FLOC * c, FLOC * (c + 1))
        # core c's Ws rows for RS chunk ch, sub-tile j: [2048ch+256c+128j, +128)
        wsb = np.stack(
            [Ws[0, CROWS * ch + 256 * c + 128 * j:
                CROWS * ch + 256 * c + 128 * (j + 1)]
             for ch in range(NCH) for j in range(2)], axis=1).astype(f16)
        m = {"catT": catT, "remT": remT, "masks": masks, "consts": consts,
             "wsb": np.ascontiguousarray(wsb)}
        for l, Wqkv, Wo in (("1", Wqkv1, Wo1), ("2", Wqkv2, Wo2),
                            ("c", Wqkvc, Woc)):
            Wq, Wk, Wv = Wqkv[:E], Wqkv[E:2 * E], Wqkv[2 * E:]
            m["wq" + l] = np.ascontiguousarray(Wq[hs].T.astype(f16))
            m["wk" + l] = np.ascontiguousarray(Wk[hs].T.astype(f16))
            m["wv" + l] = np.ascontiguousarray(Wv[hs].T.astype(f16))
            m["wo" + l] = np.ascontiguousarray(Wo[:, hs].T.astype(f16))
        m["wf1"] = np.ascontiguousarray(Wf1[fs].T.astype(f16))
        m["wf2"] = np.ascontiguousarray(Wf2[:, fs].T.astype(f16))
        # row-sums over E for the LN-identity rank-1 terms (fp32 accum)
        m["wqcr"] = Wqkvc[:E][hs].sum(axis=1).astype(f16)[None, :]
        m["wf1r"] = Wf1[fs].sum(axis=1).astype(f16)[None, :]
        in_maps.append(m)
    return in_maps


def run_device(in_maps, ncat_real, nrem_real, debug=False, trace=False):
    from concourse.bass_utils import run_bass_kernel_spmd

    key = (ncat_real, nrem_real, debug)
    if key not in _CACHE:
        _CACHE[key] = _build_device(ncat_real, nrem_real, debug=debug)
    nc = _CACHE[key]
    return run_bass_kernel_spmd(nc, in_maps, list(range(NCORES)), trace=trace)


def _kernel_impl(inputs, debug=False, trace=False):
    vision_feature = np.asarray(inputs["vision_feature"], np.float32)
    text_embed = np.asarray(inputs["text_embed"], np.float32)
    attention_mask = np.asarray(inputs["attention_mask"])

    biases_zero = all(
        not np.any(np.asarray(inputs[b]))
        for b in ("bqkv1", "bo1", "bqkv2", "bo2", "bqkvc", "boc",
                  "bf1", "bf2", "bs"))
    if (not bool(attention_mask.all())) or (not biases_zero):
        return _reference_np(**{k: np.asarray(v) for k, v in inputs.items()}), None

    t, sel_idx, rem_idx = _score_partition(vision_feature, text_embed,
                                           attention_mask)
    ncat_real = t + text_embed.shape[0]
    nrem_real = vision_feature.shape[0] - t
    kk = int(t * EXPAND)

    in_maps = _prep_in_maps(
        vision_feature, text_embed, sel_idx, rem_idx,
        _pad128(ncat_real), _pad128(nrem_real),
        np.asarray(inputs["Wqkv1"], np.float32),
        np.asarray(inputs["Wo1"], np.float32),
        np.asarray(inputs["Wqkv2"], np.float32),
        np.asarray(inputs["Wo2"], np.float32),
        np.asarray(inputs["Wqkvc"], np.float32),
        np.asarray(inputs["Woc"], np.float32),
        np.asarray(inputs["Wf1"], np.float32),
        np.asarray(inputs["Wf2"], np.float32),
        np.asarray(inputs["Ws"], np.float32))
    res = run_device(in_maps, ncat_real, nrem_real, debug=debug, trace=trace)
    ps = np.stack([np.asarray(res.results[c]["pstats"], np.float64)
                   for c in range(NCORES)]).sum(axis=0)
    s1, s2, wdot = ps[0], ps[1], ps[2]
    mean = s1 / E
    var = s2 / E - mean * mean
    rstd = 1.0 / np.sqrt(var + 1e-5)
    wssum = float(np.asarray(inputs["Ws"], np.float64).sum())
    logits = (wdot - mean * wssum) * rstd
    logits = logits[:nrem_real]
    es = (1.0 / (1.0 + np.exp(-logits.astype(np.float32))))
    ei = np.argsort(-es, kind="stable")[:kk]
    final = np.sort(np.concatenate([sel_idx, rem_idx[ei]]))
    return vision_feature[final], res


def kernel(**inputs):
    out, _ = _kernel_impl(inputs)
    return out
